# revision 16
# baseline (speedup 1.0000x reference)
"""Trainium2 Bass kernel for nn_CausalPrefixAttention (8-core SPMD), v3.1.

Changes vs v2 (119.6us):
  - cx is never loaded in natural layout: 8 XBAR DMA-transposes load cxT
    straight from HBM into SBUF, removing 64 PE transposes and 8 big
    PSUM->SBUF copies. ALL XBAR transposes share one queue: two concurrent
    XBAR DMAs on different queues corrupt each other (measured on device;
    per-16-token stripes of garbage). Regular DMAs on other queues are ok.
  - x still loads natural (bn_stats needs tokens-on-partitions); PE
    transposes it per-tile during the otherwise DMA-bound head (first PSUM
    batch needs only x tile 0), with all 8 PSUM->SBUF copies on ACT
    (idle then) and stats on DVE.
  - weights+consts packed into blob DMAs; win's q-block is a separate DMA
    so the q projection can start before the k/v blocks land.
  - sim PSUM is one [128,1024] f32 2-bank tile per j-tile (h0|h1), so exp
    is a single strided ACT instruction per j-tile instead of two.
  - causal tri-masking on gpsimd (Pool); out-projection PSUM->SBUF copies
    on DVE, keeping ACT = pure exp during attention.
  - final: both heads' 1/l in one reciprocal + one sel-matmul.
  - emission order (x-T, stats, cx-proj, q/k/v-proj, attention) matches
    DMA arrival so the in-order PE rarely stalls: the cost model halves PE
    clock for 3us after every stall.
"""

import os
import sys

for _p in ("/opt/trn_rl_repo", "/root/.axon_site/_ro/trn_rl_repo"):
    if os.path.isdir(_p) and _p not in sys.path:
        sys.path.append(_p)

import numpy as np

import concourse.mybir as mybir
import concourse.tile as tile
from concourse import bacc
from concourse.bass_utils import run_bass_kernel_spmd

F32 = mybir.dt.float32
BF16 = mybir.dt.bfloat16
AF = mybir.ActivationFunctionType
ALU = mybir.AluOpType

B, N, M, DIM, INNER, HEADS, DH = 2, 1024, 1024, 1024, 512, 8, 64
EPS = 1e-5
NT = N // 128      # token tiles per batch (8)
KC = DIM // 128    # contraction chunks (8)

# blob1 column offsets (bf16): wcx | idb | tri | pick | sel (row 0, 2x128)
B1_WCX, B1_IDB, B1_TRI, B1_PICK, B1_SEL = 0, 2048, 2176, 2304, 2432
B1_COLS = 2688
# blob2: win q-block | k-block | v-block | wo (split DMA: q early, rest later)
B2_WQ, B2_WK, B2_WV, B2_WO = 0, 1152, 2304, 3456
B2_COLS = 4480


def build_program(unroll=1, phase=2):
    nc = bacc.Bacc("TRN2", target_bir_lowering=False, debug=False)

    x_d = nc.dram_tensor("x", [N, DIM], BF16, kind="ExternalInput")
    cx_d = nc.dram_tensor("cx", [M, DIM], BF16, kind="ExternalInput")
    b1_d = nc.dram_tensor("b1", [128, B1_COLS], BF16, kind="ExternalInput")
    b2_d = nc.dram_tensor("b2", [128, B2_COLS], BF16, kind="ExternalInput")
    o_d = nc.dram_tensor("o", [N, DIM], BF16, kind="ExternalOutput")

    with tile.TileContext(nc) as tc:
        for _ in range(unroll):
            _emit(nc, tc, x_d, cx_d, b1_d, b2_d, o_d, phase)
    nc.compile()
    return nc


def _emit(nc, tc, x_d, cx_d, b1_d, b2_d, o_d, phase=2):
    from contextlib import ExitStack

    ctx = ExitStack()
    with ctx:
        wpool = ctx.enter_context(tc.tile_pool(name="wpool", bufs=1))
        projp = ctx.enter_context(tc.tile_pool(name="projp", bufs=5))
        vnp = ctx.enter_context(tc.tile_pool(name="vnp", bufs=4))
        ppool = ctx.enter_context(tc.tile_pool(name="ppool", bufs=3))
        otp = ctx.enter_context(tc.tile_pool(name="otp", bufs=2))
        ostp = ctx.enter_context(tc.tile_pool(name="ostp", bufs=2))
        tiny = ctx.enter_context(tc.tile_pool(name="tiny", bufs=8))
        consts = ctx.enter_context(tc.tile_pool(name="consts", bufs=1))

        eps_col = consts.tile([128, 1], F32)
        nc.vector.memset(eps_col, EPS)
        ones_col2 = consts.tile([128, 8], BF16)
        nc.vector.memset(ones_col2, 1.0)

        # ---- input DMA stream. HWDGE issue costs ~1.2us SEQ+HWDGE per
        # DMA, so the sync queue carries ONLY the tiny consts slice + the 8
        # XBAR transposes (which must share one queue); every regular load
        # goes through the Pool/SWDGE queue (25ns SEQ, no HWDGE slot). ----
        b1 = wpool.tile([128, B1_COLS], BF16, tag="b1")
        nc.sync.dma_start(out=b1[:, B1_IDB:], in_=b1_d[:, B1_IDB:])
        nc.gpsimd.dma_start(out=b1[:, 0:B1_IDB], in_=b1_d[:, 0:B1_IDB])
        wcx = b1[:, B1_WCX:B1_WCX + 2048].rearrange("p (c k) -> p c k", k=256)
        identb = b1[:, B1_IDB:B1_IDB + 128]
        tri = b1[:, B1_TRI:B1_TRI + 128]
        pick3 = b1[0:3, B1_PICK:B1_PICK + 128]
        sel2 = b1[0:1, B1_SEL:B1_SEL + 256]

        # cxT via XBAR DMA transpose, chunk-major (single queue — see above)
        cxT_t = wpool.tile([128, KC, M], BF16, tag="cxT")
        for c in range(KC):
            nc.sync.dma_start(out=cxT_t[:, c, :],
                              in_=cx_d[:, c * 128:(c + 1) * 128],
                              transpose=True)

        natx = ctx.enter_context(tc.tile_pool(name="natx", bufs=1))
        xnat_t = natx.tile([128, NT, DIM], BF16, tag="nat", name="xnat")
        x_r = x_d.rearrange("(t p) d -> p t d", p=128)
        for hf in range(NT):
            nc.gpsimd.dma_start(out=xnat_t[:, hf:hf + 1, :],
                                in_=x_r[:, hf:hf + 1, :])
        x_nat = [xnat_t[:, t, :] for t in range(NT)]
        t_order = tuple(range(NT))

        b2 = wpool.tile([128, B2_COLS], BF16, tag="b2")
        nc.gpsimd.dma_start(out=b2[:, 0:B2_WK], in_=b2_d[:, 0:B2_WK])
        nc.gpsimd.dma_start(out=b2[:, B2_WK:], in_=b2_d[:, B2_WK:])
        winq = b2[:, B2_WQ:B2_WQ + 1152].rearrange("p (c k) -> p c k", k=128)
        wink = b2[:, B2_WK:B2_WK + 1152].rearrange("p (c k) -> p c k", k=128)
        winv = b2[:, B2_WV:B2_WV + 1152].rearrange("p (c k) -> p c k", k=128)
        wo = b2[:, B2_WO:B2_WO + 1024]

        # stat rows: row0 = -mu, row1 = std (aug contraction), row2 = rs
        srow = consts.tile([3, N], BF16)

        kcxT = projp.tile([128, M], BF16, tag="proj", name="kcxT")
        vcxT = projp.tile([128, M], BF16, tag="proj", name="vcxT")
        qT = projp.tile([128, N], BF16, tag="proj", name="qT")
        kinT = projp.tile([128, N], BF16, tag="proj", name="kinT")
        vinT = projp.tile([128, N], BF16, tag="proj", name="vinT")
        rsb = ctx.enter_context(tc.tile_pool(name="rsb", bufs=2))
        rs_bc = [rsb.tile([128, 512], F32, tag="rsbc", name=f"rsbc{g}")
                 for g in range(2)]
        vn = [None] * 16

        phase_a = ExitStack()
        with phase_a:
            tposed = phase_a.enter_context(tc.tile_pool(name="tposed", bufs=1))
            psA = phase_a.enter_context(
                tc.tile_pool(name="psA", bufs=1, space="PSUM"))

            # ---- x transposes on PE, one x-tile per PSUM batch so the
            # first batch only needs x tile 0; copies on ACT ----
            xT = tposed.tile([128, 2, KC, 512], BF16, tag="tp", name="xT")
            stats4 = [None] * NT

            # 4 concurrent cx-projection PSUM chains (kcx/vcx x token-half),
            # fed chunk-by-chunk as the XBAR transposes land, interleaved
            # with the x transposes so PE streams at DMA arrival rate.
            cx_ps = {}

            def cx_chunk(c):
                for pj in (0, 1):
                    for gg in (0, 1):
                        sp = slice(gg * 512, (gg + 1) * 512)
                        if c == 0:
                            cx_ps[(pj, gg)] = psA.tile(
                                [128, 512], F32, tag=f"pps{2 * pj + gg}",
                                bufs=1, name=f"pps{2 * pj + gg}")
                        nc.tensor.matmul(
                            cx_ps[(pj, gg)],
                            wcx[:, c, pj * 128:(pj + 1) * 128],
                            cxT_t[:, c, sp],
                            start=(c == 0), stop=(c == KC - 1))

            for t in t_order:
                ps = psA.tile([128, 1024], BF16, tag="tps", bufs=2,
                              name="tps")
                for c in range(KC):
                    nc.tensor.transpose(
                        ps[:, c * 128:(c + 1) * 128],
                        x_nat[t][:, c * 128:(c + 1) * 128], identb)
                co = (t % 4) * 128
                nc.scalar.copy(
                    out=xT[:, t // 4, :, co:co + 128],
                    in_=ps.rearrange("p (c k) -> p c k", k=128))
                cx_chunk(t)
                # stats for this tile (DVE), concurrent with the transposes
                xt = x_nat[t]
                s4 = tiny.tile([128, 4], F32, tag="s4", name=f"s4_{t}")
                bst = tiny.tile([128, 2, 6], F32, tag="bst", name="bst")
                for half in range(2):
                    nc.vector.bn_stats(
                        out=bst[:, half, :],
                        in_=xt[:, half * 512:(half + 1) * 512])
                mv = tiny.tile([128, 2], F32, tag="mv", name="mv")
                nc.vector.bn_aggr(out=mv, in_=bst)
                nc.scalar.activation(
                    out=s4[:, 1:2], in_=mv[:, 1:2], func=AF.Sqrt, bias=eps_col)
                nc.vector.reciprocal(out=s4[:, 2:3], in_=s4[:, 1:2])
                nc.vector.tensor_scalar(
                    out=s4[:, 0:1], in0=mv[:, 0:1], scalar1=-1.0, scalar2=None,
                    op0=ALU.mult)
                s4b = tiny.tile([128, 3], BF16, tag="s4b", name="s4b")
                nc.vector.tensor_copy(out=s4b, in_=s4[:, 0:3])
                stats4[t] = s4b

            # cx-projection PSUM->SBUF copies (ACT: DVE is running stats)
            for pj, dst in ((0, kcxT), (1, vcxT)):
                for gg in (0, 1):
                    sp = slice(gg * 512, (gg + 1) * 512)
                    nc.scalar.copy(out=dst[:, sp], in_=cx_ps[(pj, gg)])

            # ---- stats rows (PE transposes are tiny; stats long done) ----
            for t in range(NT):
                ps2 = psA.tile([128, 512], BF16, tag="tpsr", bufs=2,
                               name="tpsr")
                nc.tensor.transpose(ps2[0:3, 0:128], stats4[t], identb)
                nc.vector.tensor_copy(
                    out=srow[:, t * 128:(t + 1) * 128], in_=ps2[0:3, 0:128])
            # rs broadcast tiles: pick3^T selects srow row 2 into every part
            for g in range(2):
                ps = psA.tile([128, 512], F32, tag=f"pps{g}", bufs=1,
                              name=f"pps{g}")
                nc.tensor.matmul(
                    ps, pick3, srow[:, g * 512:(g + 1) * 512],
                    start=True, stop=True)
                nc.scalar.copy(out=rs_bc[g], in_=ps)

            # v_nat tiles: 4 j's per [128, 520] tile, each j = [64 vfeat h0 |
            # ones | 64 vfeat h1 | ones] so the PV stationary is contiguous.
            def v_transpose(src, base):
                for q in range(2):
                    v_t = vnp.tile([128, 520], BF16, tag="vn",
                                   name=f"vn{base + 4 * q}")
                    for jj in range(4):
                        vn[base + 4 * q + jj] = (v_t, jj)
                    ps = psA.tile([128, 512], BF16, tag="tpsr", bufs=2,
                                  name="tpsr")
                    for jj in range(4):
                        j = 4 * q + jj
                        nc.tensor.transpose(
                            ps[:, jj * 128:(jj + 1) * 128],
                            src[:, j * 128:(j + 1) * 128], identb)
                    nc.vector.tensor_copy(
                        out=v_t.rearrange("p (a b) -> p a b", b=65)[:, :, 64:65],
                        in_=ones_col2.rearrange("p (a b) -> p a b", b=1))
                    nc.vector.tensor_copy(
                        out=v_t.rearrange("p (a b) -> p a b", b=65)[:, :, 0:64],
                        in_=ps.rearrange("p (a b) -> p a b", b=64))

            v_transpose(vcxT, 0)

            # dummy exp: forces the Exp act-table load off the attention
            # start (the load costs ~1.3us on ACT)
            junk = tiny.tile([128, 1], BF16, tag="junk", name="junk")
            nc.scalar.activation(out=junk, in_=eps_col, func=AF.Exp)

            # ---- input projections (q first); rs applied on PSUM->SBUF ----
            chain_i = [0]
            for w9, dst in ((winq, qT), (wink, kinT), (winv, vinT)):
                for g in range(2):
                    sp = slice(g * 512, (g + 1) * 512)
                    ps = psA.tile([128, 512], F32,
                                  tag=f"pps{chain_i[0] % 4}", bufs=1,
                                  name=f"pps{chain_i[0] % 4}")
                    chain_i[0] += 1
                    for c in range(KC):
                        nc.tensor.matmul(
                            ps, w9[:, c, :], xT[:, g, c, :],
                            start=(c == 0), stop=False)
                    nc.tensor.matmul(
                        ps, w9[0:2, KC, :], srow[0:2, sp],
                        start=False, stop=True)
                    nc.vector.tensor_tensor(
                        out=dst[:, sp], in0=ps, in1=rs_bc[g], op=ALU.mult)

            v_transpose(vinT, 8)

            if phase == 1:
                for t, src_t in enumerate((qT, kinT, vinT, kcxT, vcxT,
                                           qT, kinT, vinT)):
                    nc.sync.dma_start(
                        out=o_d[t * 128:(t + 1) * 128, :].bitcast(BF16),
                        in_=src_t)
                return

        # ---- attention + final projection ----
        with tc.tile_pool(name="psSim", bufs=1, space="PSUM") as psS, \
             tc.tile_pool(name="psO", bufs=1, space="PSUM") as psO, \
             tc.tile_pool(name="psF", bufs=1, space="PSUM") as psF:
            pend_final = [None]

            def final_head(g, o_ps):
                """lrec/lbc/oT chain. MUST be fully emitted before the next
                g's first PV (o_ps ring reuse is ordered by emission)."""
                lrec = [tiny.tile([1, 512], BF16, tag=f"lr{h}", bufs=2,
                                  name=f"lr{h}") for h in (0, 1)]
                with nc.allow_low_precision(reason="1/l in bf16 is plenty"):
                    for h in (0, 1):
                        nc.vector.tensor_copy(out=lrec[h],
                                              in_=o_ps[h][64:65, :])
                        nc.vector.reciprocal(out=lrec[h], in_=lrec[h])
                lbc_ps = psF.tile([128, 512], F32, tag="fin0", bufs=1,
                                  name="lbc")
                for h in (0, 1):
                    nc.tensor.matmul(
                        lbc_ps, sel2[:, 128 * h:128 * h + 128], lrec[h],
                        start=(h == 0), stop=(h == 1))
                lbc = tiny.tile([128, 512], F32, tag="lbc", bufs=2, name="lbc")
                nc.vector.tensor_copy(out=lbc, in_=lbc_ps)
                oT = otp.tile([128, 512], BF16, tag="oT")
                for h in (0, 1):
                    nc.vector.tensor_tensor(
                        out=oT[64 * h:64 * h + 64, :], in0=o_ps[h][0:64, :],
                        in1=lbc[64 * h:64 * h + 64, :], op=ALU.mult)
                pend_final[0] = None
                return oT

            def fin_tile(g, oT, t, tail):
                """Out-projection + store for one 128-token tile."""
                o_r = o_d.rearrange("(t p) d -> p t d", p=128)
                ost = ostp.tile([128, 1, DIM], BF16, tag="ost")
                for half in range(2):
                    wsp = slice(half * 512, (half + 1) * 512)
                    fp = psF.tile([128, 512], F32, tag=f"fin{half}",
                                  bufs=1, name=f"fin{half}")
                    nc.tensor.matmul(
                        fp, oT[:, t * 128:(t + 1) * 128], wo[:, wsp],
                        start=True, stop=True)
                    # at the tail ACT is idle (no more exp) — alternate
                    if tail and half == 1:
                        nc.scalar.copy(out=ost[:, 0, wsp], in_=fp)
                    else:
                        nc.vector.tensor_copy(out=ost[:, 0, wsp], in_=fp)
                eng = nc.sync if t % 2 == 0 else nc.scalar
                eng.dma_start(out=o_r[:, g * 4 + t:g * 4 + t + 1, :], in_=ost)

            prev_g = [None]
            for g in (0, 1):
                # j order: cx0..cx6, in0.., cx7 (start/stop on full spans)
                j_list = [("cx", j) for j in range(7)]
                j_list += [("in", j) for j in range(4 * g + 4)]
                j_list.append(("cx", 7))
                n_j = len(j_list)
                o_ps = [psO.tile([128, 512], F32, tag=f"o{h}", name=f"ops{h}")
                        for h in (0, 1)]

                def j_meta(idx, g=g, j_list=j_list):
                    src, j = j_list[idx]
                    if src == "cx":
                        return kcxT, j, j, 0, False
                    off = max(0, 128 * (j - 4 * g))
                    return kinT, j, 8 + j, off, j >= 4 * g

                sims = [None] * n_j

                def emit_sim(idx, j_meta=j_meta, sims=sims, g=g):
                    kT, j, jg, off, diag = j_meta(idx)
                    ps = psS.tile([128, 1024], F32, tag="sim", bufs=2,
                                  name="sim")
                    for h in (0, 1):
                        hsl = slice(64 * h, 64 * h + 64)
                        nc.tensor.matmul(
                            ps[:, 512 * h + off:512 * (h + 1)],
                            kT[hsl, j * 128:(j + 1) * 128],
                            qT[hsl, g * 512 + off:(g + 1) * 512],
                            start=True, stop=True)
                    sims[idx] = ps

                # software pipeline: sim for j+1 is emitted before PV of j so
                # the in-order PE computes the next sim while ACT runs exp.
                emit_sim(0)
                # previous g's final: the o_ps-reading head goes here (before
                # this g's first PV); the 4 out-projection tiles interleave
                # into the j loop so their PE matmuls fill exp-wait gaps.
                fin_steps = []
                if pend_final[0] is not None:
                    oT_prev = final_head(prev_g[0], pend_final[0])
                    fin_steps = [(prev_g[0], oT_prev, t) for t in range(4)]
                for idx in range(n_j):
                    if idx + 1 < n_j:
                        emit_sim(idx + 1)
                    if fin_steps and idx >= 2 and idx % 2 == 0:
                        pg, oTp, t = fin_steps.pop(0)
                        fin_tile(pg, oTp, t, tail=False)
                    kT, j, jg, off, diag = j_meta(idx)
                    p_t = ppool.tile([128, 1024], BF16, tag="p", name="p")
                    ps3 = sims[idx].rearrange("p (h t) -> p h t", h=2)
                    p3 = p_t.rearrange("p (h t) -> p h t", h=2)
                    nc.scalar.activation(
                        out=p3[:, :, off:512], in_=ps3[:, :, off:512],
                        func=AF.Exp)
                    if diag:
                        for h in (0, 1):
                            nc.gpsimd.tensor_tensor(
                                out=p_t[:, 512 * h + off:512 * h + off + 128],
                                in0=p_t[:, 512 * h + off:512 * h + off + 128],
                                in1=tri, op=ALU.mult)
                    sims[idx] = None
                    v_t, jj = vn[jg]
                    for h in (0, 1):
                        nc.tensor.matmul(
                            o_ps[h][0:65, off:512],
                            v_t[:, 130 * jj + 65 * h:130 * jj + 65 * h + 65],
                            p_t[:, 512 * h + off:512 * (h + 1)],
                            start=(idx == 0), stop=(idx == n_j - 1))
                for pg, oTp, t in fin_steps:
                    fin_tile(pg, oTp, t, tail=False)
                pend_final[0] = o_ps
                prev_g[0] = g
            oT_last = final_head(1, pend_final[0])
            for t in range(4):
                fin_tile(1, oT_last, t, tail=True)


_NC_CACHE = None


def _get_nc():
    global _NC_CACHE
    if _NC_CACHE is None:
        _NC_CACHE = build_program()
    return _NC_CACHE


def make_in_maps(x, context, gamma, beta, Wq, Wkv, Wo, bo):
    import ml_dtypes
    BF = ml_dtypes.bfloat16
    x = np.asarray(x, np.float32)
    context = np.asarray(context, np.float32)
    gamma = np.asarray(gamma, np.float32)
    beta = np.asarray(beta, np.float32)
    Wq = np.asarray(Wq, np.float32)
    Wkv = np.asarray(Wkv, np.float32)
    Wo = np.asarray(Wo, np.float32)

    s = DH ** -0.5
    in_maps = []
    for core in range(8):
        b, hg = divmod(core, 4)
        cols = slice(128 * hg, 128 * hg + 128)
        wq = Wq[:, cols] * gamma[:, None] * s
        uq = wq.sum(0)
        bq = beta @ Wq[:, cols] * s
        wk = Wkv[:, :INNER][:, cols] * gamma[:, None]
        uk = wk.sum(0)
        bk = beta @ Wkv[:, :INNER][:, cols]
        wv = Wkv[:, INNER:][:, cols] * gamma[:, None]
        uv = wv.sum(0)
        bv = beta @ Wkv[:, INNER:][:, cols]

        # per-projection 9-chunk blocks (chunk 8 = aug rows u, b)
        def blk(w, u, bvec):
            out = np.zeros((128, KC + 1, 128), np.float32)
            for c in range(KC):
                out[:, c, :] = w[128 * c:128 * c + 128]
            out[0, KC, :] = u
            out[1, KC, :] = bvec
            return out.reshape(128, 1152)

        wcx = np.zeros((128, KC, 256), np.float32)
        for c in range(KC):
            rows = slice(128 * c, 128 * c + 128)
            wcx[:, c, 0:128] = Wkv[:, :INNER][rows, cols]
            wcx[:, c, 128:256] = Wkv[:, INNER:][rows, cols]

        b1 = np.zeros((128, B1_COLS), np.float32)
        b1[:, B1_WCX:B1_WCX + 2048] = wcx.reshape(128, 2048)
        b1[:, B1_IDB:B1_IDB + 128] = np.eye(128, dtype=np.float32)
        b1[:, B1_TRI:B1_TRI + 128] = np.tril(np.ones((128, 128), np.float32)).T
        b1[2, B1_PICK:B1_PICK + 128] = 1.0
        b1[0, B1_SEL:B1_SEL + 64] = 1.0
        b1[0, B1_SEL + 192:B1_SEL + 256] = 1.0

        b2 = np.zeros((128, B2_COLS), np.float32)
        b2[:, B2_WQ:B2_WQ + 1152] = blk(wq, uq, bq)
        b2[:, B2_WK:B2_WK + 1152] = blk(wk, uk, bk)
        b2[:, B2_WV:B2_WV + 1152] = blk(wv, uv, bv)
        b2[:, B2_WO:B2_WO + 1024] = Wo[cols, :]

        in_maps.append({
            "x": np.ascontiguousarray(x[b]).astype(BF),
            "cx": np.ascontiguousarray(context[b]).astype(BF),
            "b1": b1.astype(BF),
            "b2": b2.astype(BF),
        })
    return in_maps


def assemble(results, bo):
    bo = np.asarray(bo, np.float32)
    out = np.zeros((B, N, DIM), np.float32)
    for core in range(8):
        b = core // 4
        out[b] += results[core]["o"].astype(np.float32)
    out += bo[None, None, :]
    return out


def kernel(x, context, gamma, beta, Wq, Wkv, Wo, bo):
    nc = _get_nc()
    in_maps = make_in_maps(x, context, gamma, beta, Wq, Wkv, Wo, bo)
    res = run_bass_kernel_spmd(nc, in_maps, list(range(8)))
    return assemble(res.results, bo)


# revision 17
# speedup vs baseline: 1.2354x; 1.2354x over previous
"""Trainium2 Bass kernel for nn_CausalPrefixAttention (8-core SPMD), v3.1.

Changes vs v2 (119.6us):
  - cx is never loaded in natural layout: 8 XBAR DMA-transposes load cxT
    straight from HBM into SBUF, removing 64 PE transposes and 8 big
    PSUM->SBUF copies. ALL XBAR transposes share one queue: two concurrent
    XBAR DMAs on different queues corrupt each other (measured on device;
    per-16-token stripes of garbage). Regular DMAs on other queues are ok.
  - x still loads natural (bn_stats needs tokens-on-partitions); PE
    transposes it per-tile during the otherwise DMA-bound head (first PSUM
    batch needs only x tile 0), with all 8 PSUM->SBUF copies on ACT
    (idle then) and stats on DVE.
  - weights+consts packed into blob DMAs; win's q-block is a separate DMA
    so the q projection can start before the k/v blocks land.
  - sim PSUM is one [128,1024] f32 2-bank tile per j-tile (h0|h1), so exp
    is a single strided ACT instruction per j-tile instead of two.
  - causal tri-masking on gpsimd (Pool); out-projection PSUM->SBUF copies
    on DVE, keeping ACT = pure exp during attention.
  - final: both heads' 1/l in one reciprocal + one sel-matmul.
  - emission order (x-T, stats, cx-proj, q/k/v-proj, attention) matches
    DMA arrival so the in-order PE rarely stalls: the cost model halves PE
    clock for 3us after every stall.
"""

import os
import sys

for _p in ("/opt/trn_rl_repo", "/root/.axon_site/_ro/trn_rl_repo"):
    if os.path.isdir(_p) and _p not in sys.path:
        sys.path.append(_p)

import numpy as np

import concourse.mybir as mybir
import concourse.tile as tile
from concourse import bacc
from concourse.bass_utils import run_bass_kernel_spmd

F32 = mybir.dt.float32
BF16 = mybir.dt.bfloat16
AF = mybir.ActivationFunctionType
ALU = mybir.AluOpType

B, N, M, DIM, INNER, HEADS, DH = 2, 1024, 1024, 1024, 512, 8, 64
EPS = 1e-5
NT = N // 128      # token tiles per batch (8)
KC = DIM // 128    # contraction chunks (8)

# blob1 column offsets (bf16): wcx | idb | tri | pick | sel (row 0, 2x128)
B1_WCX, B1_IDB, B1_TRI, B1_PICK, B1_SEL = 0, 2048, 2176, 2304, 2432
B1_COLS = 2688
# blob2: win q-block | k-block | v-block | wo (split DMA: q early, rest later)
B2_WQ, B2_WK, B2_WV, B2_WO = 0, 1152, 2304, 3456
B2_COLS = 4480


def build_program(unroll=1, phase=2):
    nc = bacc.Bacc("TRN2", target_bir_lowering=False, debug=False)

    x_d = nc.dram_tensor("x", [N, DIM], BF16, kind="ExternalInput")
    cx_d = nc.dram_tensor("cx", [M, DIM], BF16, kind="ExternalInput")
    b1_d = nc.dram_tensor("b1", [128, B1_COLS], BF16, kind="ExternalInput")
    b2_d = nc.dram_tensor("b2", [128, B2_COLS], BF16, kind="ExternalInput")
    o_d = nc.dram_tensor("o", [N, DIM], BF16, kind="ExternalOutput")

    with tile.TileContext(nc) as tc:
        for _ in range(unroll):
            _emit(nc, tc, x_d, cx_d, b1_d, b2_d, o_d, phase)
    nc.compile()
    return nc


def _emit(nc, tc, x_d, cx_d, b1_d, b2_d, o_d, phase=2):
    from contextlib import ExitStack

    ctx = ExitStack()
    with ctx:
        wpool = ctx.enter_context(tc.tile_pool(name="wpool", bufs=1))
        projp = ctx.enter_context(tc.tile_pool(name="projp", bufs=5))
        vnp = ctx.enter_context(tc.tile_pool(name="vnp", bufs=4))
        ppool = ctx.enter_context(tc.tile_pool(name="ppool", bufs=3))
        otp = ctx.enter_context(tc.tile_pool(name="otp", bufs=2))
        ostp = ctx.enter_context(tc.tile_pool(name="ostp", bufs=2))
        tiny = ctx.enter_context(tc.tile_pool(name="tiny", bufs=8))
        consts = ctx.enter_context(tc.tile_pool(name="consts", bufs=1))

        eps_col = consts.tile([128, 1], F32)
        nc.vector.memset(eps_col, EPS)
        ones_col2 = consts.tile([128, 8], BF16)
        nc.vector.memset(ones_col2, 1.0)

        # ---- input DMA stream. sync: b1, x-even, cxT transposes;
        # scalar: x-odd, win/wo. ----
        b1 = wpool.tile([128, B1_COLS], BF16, tag="b1")
        nc.sync.dma_start(out=b1, in_=b1_d[:])
        wcx = b1[:, B1_WCX:B1_WCX + 2048].rearrange("p (c k) -> p c k", k=256)
        identb = b1[:, B1_IDB:B1_IDB + 128]
        tri = b1[:, B1_TRI:B1_TRI + 128]
        pick3 = b1[0:3, B1_PICK:B1_PICK + 128]
        sel2 = b1[0:1, B1_SEL:B1_SEL + 256]

        natx = ctx.enter_context(tc.tile_pool(name="natx", bufs=1))
        xnat_t = natx.tile([128, NT, DIM], BF16, tag="nat", name="xnat")
        x_r = x_d.rearrange("(t p) d -> p t d", p=128)
        for hf in range(NT):
            eng = nc.sync if hf % 2 == 0 else nc.scalar
            eng.dma_start(out=xnat_t[:, hf:hf + 1, :], in_=x_r[:, hf:hf + 1, :])
        x_nat = [xnat_t[:, t, :] for t in range(NT)]
        t_order = tuple(range(NT))

        # cxT via XBAR DMA transpose, chunk-major (single queue — see above)
        cxT_t = wpool.tile([128, KC, M], BF16, tag="cxT")
        for c in range(KC):
            nc.sync.dma_start(out=cxT_t[:, c, :],
                              in_=cx_d[:, c * 128:(c + 1) * 128],
                              transpose=True)

        b2 = wpool.tile([128, B2_COLS], BF16, tag="b2")
        nc.scalar.dma_start(out=b2[:, 0:B2_WK], in_=b2_d[:, 0:B2_WK])
        nc.scalar.dma_start(out=b2[:, B2_WK:], in_=b2_d[:, B2_WK:])
        winq = b2[:, B2_WQ:B2_WQ + 1152].rearrange("p (c k) -> p c k", k=128)
        wink = b2[:, B2_WK:B2_WK + 1152].rearrange("p (c k) -> p c k", k=128)
        winv = b2[:, B2_WV:B2_WV + 1152].rearrange("p (c k) -> p c k", k=128)
        wo = b2[:, B2_WO:B2_WO + 1024]

        # stat rows: row0 = -mu, row1 = std (aug contraction), row2 = rs
        srow = consts.tile([3, N], BF16)

        kcxT = projp.tile([128, M], BF16, tag="proj", name="kcxT")
        vcxT = projp.tile([128, M], BF16, tag="proj", name="vcxT")
        qT = projp.tile([128, N], BF16, tag="proj", name="qT")
        kinT = projp.tile([128, N], BF16, tag="proj", name="kinT")
        vinT = projp.tile([128, N], BF16, tag="proj", name="vinT")
        rsb = ctx.enter_context(tc.tile_pool(name="rsb", bufs=2))
        rs_bc = [rsb.tile([128, 512], F32, tag="rsbc", name=f"rsbc{g}")
                 for g in range(2)]
        vn = [None] * 16

        phase_a = ExitStack()
        with phase_a:
            tposed = phase_a.enter_context(tc.tile_pool(name="tposed", bufs=1))
            psA = phase_a.enter_context(
                tc.tile_pool(name="psA", bufs=1, space="PSUM"))

            # ---- x transposes on PE, one x-tile per PSUM batch so the
            # first batch only needs x tile 0; copies on ACT ----
            xT = tposed.tile([128, 2, KC, 512], BF16, tag="tp", name="xT")
            stats4 = [None] * NT

            for t in t_order:
                ps = psA.tile([128, 1024], BF16, tag="tps", bufs=2,
                              name="tps")
                for c in range(KC):
                    nc.tensor.transpose(
                        ps[:, c * 128:(c + 1) * 128],
                        x_nat[t][:, c * 128:(c + 1) * 128], identb)
                co = (t % 4) * 128
                nc.scalar.copy(
                    out=xT[:, t // 4, :, co:co + 128],
                    in_=ps.rearrange("p (c k) -> p c k", k=128))
                # stats for this tile (DVE), concurrent with the transposes
                xt = x_nat[t]
                s4 = tiny.tile([128, 4], F32, tag="s4", name=f"s4_{t}")
                bst = tiny.tile([128, 2, 6], F32, tag="bst", name="bst")
                for half in range(2):
                    nc.vector.bn_stats(
                        out=bst[:, half, :],
                        in_=xt[:, half * 512:(half + 1) * 512])
                mv = tiny.tile([128, 2], F32, tag="mv", name="mv")
                nc.vector.bn_aggr(out=mv, in_=bst)
                nc.scalar.activation(
                    out=s4[:, 1:2], in_=mv[:, 1:2], func=AF.Sqrt, bias=eps_col)
                nc.vector.reciprocal(out=s4[:, 2:3], in_=s4[:, 1:2])
                nc.vector.tensor_scalar(
                    out=s4[:, 0:1], in0=mv[:, 0:1], scalar1=-1.0, scalar2=None,
                    op0=ALU.mult)
                s4b = tiny.tile([128, 3], BF16, tag="s4b", name="s4b")
                nc.vector.tensor_copy(out=s4b, in_=s4[:, 0:3])
                stats4[t] = s4b

            # ---- context projections (cxT streamed by the DMA queue);
            # copies on ACT: DVE is running the stats chain ----
            ci = [0]
            for pj, dst in ((0, kcxT), (1, vcxT)):
                for gg in (0, 1):
                    sp = slice(gg * 512, (gg + 1) * 512)
                    ps = psA.tile([128, 512], F32, tag=f"pps{ci[0] % 4}",
                                  bufs=1, name=f"pps{ci[0] % 4}")
                    ci[0] += 1
                    for c in range(KC):
                        nc.tensor.matmul(
                            ps, wcx[:, c, pj * 128:(pj + 1) * 128],
                            cxT_t[:, c, sp],
                            start=(c == 0), stop=(c == KC - 1))
                    nc.scalar.copy(out=dst[:, sp], in_=ps)

            # ---- stats rows (PE transposes are tiny; stats long done) ----
            for t in range(NT):
                ps2 = psA.tile([128, 512], BF16, tag="tpsr", bufs=2,
                               name="tpsr")
                nc.tensor.transpose(ps2[0:3, 0:128], stats4[t], identb)
                nc.vector.tensor_copy(
                    out=srow[:, t * 128:(t + 1) * 128], in_=ps2[0:3, 0:128])
            # rs broadcast tiles: pick3^T selects srow row 2 into every part
            for g in range(2):
                ps = psA.tile([128, 512], F32, tag=f"pps{g}", bufs=1,
                              name=f"pps{g}")
                nc.tensor.matmul(
                    ps, pick3, srow[:, g * 512:(g + 1) * 512],
                    start=True, stop=True)
                nc.scalar.copy(out=rs_bc[g], in_=ps)

            # v_nat tiles: 4 j's per [128, 520] tile, each j = [64 vfeat h0 |
            # ones | 64 vfeat h1 | ones] so the PV stationary is contiguous.
            def v_transpose(src, base):
                for q in range(2):
                    v_t = vnp.tile([128, 520], BF16, tag="vn",
                                   name=f"vn{base + 4 * q}")
                    for jj in range(4):
                        vn[base + 4 * q + jj] = (v_t, jj)
                    ps = psA.tile([128, 512], BF16, tag="tpsr", bufs=2,
                                  name="tpsr")
                    for jj in range(4):
                        j = 4 * q + jj
                        nc.tensor.transpose(
                            ps[:, jj * 128:(jj + 1) * 128],
                            src[:, j * 128:(j + 1) * 128], identb)
                    nc.vector.tensor_copy(
                        out=v_t.rearrange("p (a b) -> p a b", b=65)[:, :, 64:65],
                        in_=ones_col2.rearrange("p (a b) -> p a b", b=1))
                    nc.vector.tensor_copy(
                        out=v_t.rearrange("p (a b) -> p a b", b=65)[:, :, 0:64],
                        in_=ps.rearrange("p (a b) -> p a b", b=64))

            v_transpose(vcxT, 0)

            # dummy exp: forces the Exp act-table load off the attention
            # start (the load costs ~1.3us on ACT)
            junk = tiny.tile([128, 1], BF16, tag="junk", name="junk")
            nc.scalar.activation(out=junk, in_=eps_col, func=AF.Exp)

            # ---- input projections (q first); rs applied on PSUM->SBUF ----
            chain_i = [0]
            for w9, dst in ((winq, qT), (wink, kinT), (winv, vinT)):
                for g in range(2):
                    sp = slice(g * 512, (g + 1) * 512)
                    ps = psA.tile([128, 512], F32,
                                  tag=f"pps{chain_i[0] % 4}", bufs=1,
                                  name=f"pps{chain_i[0] % 4}")
                    chain_i[0] += 1
                    for c in range(KC):
                        nc.tensor.matmul(
                            ps, w9[:, c, :], xT[:, g, c, :],
                            start=(c == 0), stop=False)
                    nc.tensor.matmul(
                        ps, w9[0:2, KC, :], srow[0:2, sp],
                        start=False, stop=True)
                    nc.vector.tensor_tensor(
                        out=dst[:, sp], in0=ps, in1=rs_bc[g], op=ALU.mult)

            v_transpose(vinT, 8)

            if phase == 1:
                for t, src_t in enumerate((qT, kinT, vinT, kcxT, vcxT,
                                           qT, kinT, vinT)):
                    nc.sync.dma_start(
                        out=o_d[t * 128:(t + 1) * 128, :].bitcast(BF16),
                        in_=src_t)
                return

        # ---- attention + final projection ----
        with tc.tile_pool(name="psSim", bufs=1, space="PSUM") as psS, \
             tc.tile_pool(name="psO", bufs=1, space="PSUM") as psO, \
             tc.tile_pool(name="psF", bufs=1, space="PSUM") as psF:
            pend_final = [None]

            def final_head(g, o_ps):
                """lrec/lbc/oT chain. MUST be fully emitted before the next
                g's first PV (o_ps ring reuse is ordered by emission)."""
                lrec = [tiny.tile([1, 512], BF16, tag=f"lr{h}", bufs=2,
                                  name=f"lr{h}") for h in (0, 1)]
                with nc.allow_low_precision(reason="1/l in bf16 is plenty"):
                    for h in (0, 1):
                        nc.vector.tensor_copy(out=lrec[h],
                                              in_=o_ps[h][64:65, :])
                        nc.vector.reciprocal(out=lrec[h], in_=lrec[h])
                lbc_ps = psF.tile([128, 512], F32, tag="fin0", bufs=1,
                                  name="lbc")
                for h in (0, 1):
                    nc.tensor.matmul(
                        lbc_ps, sel2[:, 128 * h:128 * h + 128], lrec[h],
                        start=(h == 0), stop=(h == 1))
                lbc = tiny.tile([128, 512], F32, tag="lbc", bufs=2, name="lbc")
                nc.vector.tensor_copy(out=lbc, in_=lbc_ps)
                oT = otp.tile([128, 512], BF16, tag="oT")
                for h in (0, 1):
                    nc.vector.tensor_tensor(
                        out=oT[64 * h:64 * h + 64, :], in0=o_ps[h][0:64, :],
                        in1=lbc[64 * h:64 * h + 64, :], op=ALU.mult)
                pend_final[0] = None
                return oT

            def fin_tile(g, oT, t, tail):
                """Out-projection + store for one 128-token tile."""
                o_r = o_d.rearrange("(t p) d -> p t d", p=128)
                ost = ostp.tile([128, 1, DIM], BF16, tag="ost")
                for half in range(2):
                    wsp = slice(half * 512, (half + 1) * 512)
                    fp = psF.tile([128, 512], F32, tag=f"fin{half}",
                                  bufs=1, name=f"fin{half}")
                    nc.tensor.matmul(
                        fp, oT[:, t * 128:(t + 1) * 128], wo[:, wsp],
                        start=True, stop=True)
                    # at the tail ACT is idle (no more exp) — alternate
                    if tail and half == 1:
                        nc.scalar.copy(out=ost[:, 0, wsp], in_=fp)
                    else:
                        nc.vector.tensor_copy(out=ost[:, 0, wsp], in_=fp)
                eng = nc.sync if t % 2 == 0 else nc.scalar
                eng.dma_start(out=o_r[:, g * 4 + t:g * 4 + t + 1, :], in_=ost)

            prev_g = [None]
            for g in (0, 1):
                # j order: cx0..cx6, in0.., cx7 (start/stop on full spans)
                j_list = [("cx", j) for j in range(7)]
                j_list += [("in", j) for j in range(4 * g + 4)]
                j_list.append(("cx", 7))
                n_j = len(j_list)
                o_ps = [psO.tile([128, 512], F32, tag=f"o{h}", name=f"ops{h}")
                        for h in (0, 1)]

                def j_meta(idx, g=g, j_list=j_list):
                    src, j = j_list[idx]
                    if src == "cx":
                        return kcxT, j, j, 0, False
                    off = max(0, 128 * (j - 4 * g))
                    return kinT, j, 8 + j, off, j >= 4 * g

                sims = [None] * n_j

                def emit_sim(idx, j_meta=j_meta, sims=sims, g=g):
                    kT, j, jg, off, diag = j_meta(idx)
                    ps = psS.tile([128, 1024], F32, tag="sim", bufs=2,
                                  name="sim")
                    for h in (0, 1):
                        hsl = slice(64 * h, 64 * h + 64)
                        nc.tensor.matmul(
                            ps[:, 512 * h + off:512 * (h + 1)],
                            kT[hsl, j * 128:(j + 1) * 128],
                            qT[hsl, g * 512 + off:(g + 1) * 512],
                            start=True, stop=True)
                    sims[idx] = ps

                # software pipeline: sim for j+1 is emitted before PV of j so
                # the in-order PE computes the next sim while ACT runs exp.
                emit_sim(0)
                # previous g's final: the o_ps-reading head goes here (before
                # this g's first PV); the 4 out-projection tiles interleave
                # into the j loop so their PE matmuls fill exp-wait gaps.
                fin_steps = []
                if pend_final[0] is not None:
                    oT_prev = final_head(prev_g[0], pend_final[0])
                    fin_steps = [(prev_g[0], oT_prev, t) for t in range(4)]
                for idx in range(n_j):
                    if idx + 1 < n_j:
                        emit_sim(idx + 1)
                    if fin_steps and idx >= 2 and idx % 2 == 0:
                        pg, oTp, t = fin_steps.pop(0)
                        fin_tile(pg, oTp, t, tail=False)
                    kT, j, jg, off, diag = j_meta(idx)
                    p_t = ppool.tile([128, 1024], BF16, tag="p", name="p")
                    ps3 = sims[idx].rearrange("p (h t) -> p h t", h=2)
                    p3 = p_t.rearrange("p (h t) -> p h t", h=2)
                    nc.scalar.activation(
                        out=p3[:, :, off:512], in_=ps3[:, :, off:512],
                        func=AF.Exp)
                    if diag:
                        for h in (0, 1):
                            nc.gpsimd.tensor_tensor(
                                out=p_t[:, 512 * h + off:512 * h + off + 128],
                                in0=p_t[:, 512 * h + off:512 * h + off + 128],
                                in1=tri, op=ALU.mult)
                    sims[idx] = None
                    v_t, jj = vn[jg]
                    for h in (0, 1):
                        nc.tensor.matmul(
                            o_ps[h][0:65, off:512],
                            v_t[:, 130 * jj + 65 * h:130 * jj + 65 * h + 65],
                            p_t[:, 512 * h + off:512 * (h + 1)],
                            start=(idx == 0), stop=(idx == n_j - 1))
                for pg, oTp, t in fin_steps:
                    fin_tile(pg, oTp, t, tail=False)
                pend_final[0] = o_ps
                prev_g[0] = g
            oT_last = final_head(1, pend_final[0])
            for t in range(4):
                fin_tile(1, oT_last, t, tail=True)


_NC_CACHE = None


def _get_nc():
    global _NC_CACHE
    if _NC_CACHE is None:
        _NC_CACHE = build_program()
    return _NC_CACHE


def make_in_maps(x, context, gamma, beta, Wq, Wkv, Wo, bo):
    import ml_dtypes
    BF = ml_dtypes.bfloat16
    x = np.asarray(x, np.float32)
    context = np.asarray(context, np.float32)
    gamma = np.asarray(gamma, np.float32)
    beta = np.asarray(beta, np.float32)
    Wq = np.asarray(Wq, np.float32)
    Wkv = np.asarray(Wkv, np.float32)
    Wo = np.asarray(Wo, np.float32)

    s = DH ** -0.5
    in_maps = []
    for core in range(8):
        b, hg = divmod(core, 4)
        cols = slice(128 * hg, 128 * hg + 128)
        wq = Wq[:, cols] * gamma[:, None] * s
        uq = wq.sum(0)
        bq = beta @ Wq[:, cols] * s
        wk = Wkv[:, :INNER][:, cols] * gamma[:, None]
        uk = wk.sum(0)
        bk = beta @ Wkv[:, :INNER][:, cols]
        wv = Wkv[:, INNER:][:, cols] * gamma[:, None]
        uv = wv.sum(0)
        bv = beta @ Wkv[:, INNER:][:, cols]

        # per-projection 9-chunk blocks (chunk 8 = aug rows u, b)
        def blk(w, u, bvec):
            out = np.zeros((128, KC + 1, 128), np.float32)
            for c in range(KC):
                out[:, c, :] = w[128 * c:128 * c + 128]
            out[0, KC, :] = u
            out[1, KC, :] = bvec
            return out.reshape(128, 1152)

        wcx = np.zeros((128, KC, 256), np.float32)
        for c in range(KC):
            rows = slice(128 * c, 128 * c + 128)
            wcx[:, c, 0:128] = Wkv[:, :INNER][rows, cols]
            wcx[:, c, 128:256] = Wkv[:, INNER:][rows, cols]

        b1 = np.zeros((128, B1_COLS), np.float32)
        b1[:, B1_WCX:B1_WCX + 2048] = wcx.reshape(128, 2048)
        b1[:, B1_IDB:B1_IDB + 128] = np.eye(128, dtype=np.float32)
        b1[:, B1_TRI:B1_TRI + 128] = np.tril(np.ones((128, 128), np.float32)).T
        b1[2, B1_PICK:B1_PICK + 128] = 1.0
        b1[0, B1_SEL:B1_SEL + 64] = 1.0
        b1[0, B1_SEL + 192:B1_SEL + 256] = 1.0

        b2 = np.zeros((128, B2_COLS), np.float32)
        b2[:, B2_WQ:B2_WQ + 1152] = blk(wq, uq, bq)
        b2[:, B2_WK:B2_WK + 1152] = blk(wk, uk, bk)
        b2[:, B2_WV:B2_WV + 1152] = blk(wv, uv, bv)
        b2[:, B2_WO:B2_WO + 1024] = Wo[cols, :]

        in_maps.append({
            "x": np.ascontiguousarray(x[b]).astype(BF),
            "cx": np.ascontiguousarray(context[b]).astype(BF),
            "b1": b1.astype(BF),
            "b2": b2.astype(BF),
        })
    return in_maps


def assemble(results, bo):
    bo = np.asarray(bo, np.float32)
    out = np.zeros((B, N, DIM), np.float32)
    for core in range(8):
        b = core // 4
        out[b] += results[core]["o"].astype(np.float32)
    out += bo[None, None, :]
    return out


def kernel(x, context, gamma, beta, Wq, Wkv, Wo, bo):
    nc = _get_nc()
    in_maps = make_in_maps(x, context, gamma, beta, Wq, Wkv, Wo, bo)
    res = run_bass_kernel_spmd(nc, in_maps, list(range(8)))
    return assemble(res.results, bo)


# revision 18
# speedup vs baseline: 1.2758x; 1.0327x over previous
"""Trainium2 Bass kernel for nn_CausalPrefixAttention (8-core SPMD), v3.1.

Changes vs v2 (119.6us):
  - cx is never loaded in natural layout: 8 XBAR DMA-transposes load cxT
    straight from HBM into SBUF, removing 64 PE transposes and 8 big
    PSUM->SBUF copies. ALL XBAR transposes share one queue: two concurrent
    XBAR DMAs on different queues corrupt each other (measured on device;
    per-16-token stripes of garbage). Regular DMAs on other queues are ok.
  - x still loads natural (bn_stats needs tokens-on-partitions); PE
    transposes it per-tile during the otherwise DMA-bound head (first PSUM
    batch needs only x tile 0), with all 8 PSUM->SBUF copies on ACT
    (idle then) and stats on DVE.
  - weights+consts packed into blob DMAs; win's q-block is a separate DMA
    so the q projection can start before the k/v blocks land.
  - sim PSUM is one [128,1024] f32 2-bank tile per j-tile (h0|h1), so exp
    is a single strided ACT instruction per j-tile instead of two.
  - causal tri-masking on gpsimd (Pool); out-projection PSUM->SBUF copies
    on DVE, keeping ACT = pure exp during attention.
  - final: both heads' 1/l in one reciprocal + one sel-matmul.
  - emission order (x-T, stats, cx-proj, q/k/v-proj, attention) matches
    DMA arrival so the in-order PE rarely stalls: the cost model halves PE
    clock for 3us after every stall.
"""

import os
import sys

for _p in ("/opt/trn_rl_repo", "/root/.axon_site/_ro/trn_rl_repo"):
    if os.path.isdir(_p) and _p not in sys.path:
        sys.path.append(_p)

import numpy as np

import concourse.mybir as mybir
import concourse.tile as tile
from concourse import bacc
from concourse.bass_utils import run_bass_kernel_spmd

F32 = mybir.dt.float32
BF16 = mybir.dt.bfloat16
AF = mybir.ActivationFunctionType
ALU = mybir.AluOpType

B, N, M, DIM, INNER, HEADS, DH = 2, 1024, 1024, 1024, 512, 8, 64
EPS = 1e-5
NT = N // 128      # token tiles per batch (8)
KC = DIM // 128    # contraction chunks (8)

# blob1 column offsets (bf16): wcx | idb | tri | pick | sel (row 0, 2x128)
B1_WCX, B1_IDB, B1_TRI, B1_PICK, B1_SEL = 0, 2048, 2176, 2304, 2432
B1_COLS = 2688
# blob2: win q-block | k-block | v-block | wo (split DMA: q early, rest later)
B2_WQ, B2_WK, B2_WV, B2_WO = 0, 1152, 2304, 3456
B2_COLS = 4480


def build_program(unroll=1, phase=2):
    nc = bacc.Bacc("TRN2", target_bir_lowering=False, debug=False)

    x_d = nc.dram_tensor("x", [N, DIM], BF16, kind="ExternalInput")
    cx_d = nc.dram_tensor("cx", [M, DIM], BF16, kind="ExternalInput")
    b1_d = nc.dram_tensor("b1", [128, B1_COLS], BF16, kind="ExternalInput")
    b2_d = nc.dram_tensor("b2", [128, B2_COLS], BF16, kind="ExternalInput")
    o_d = nc.dram_tensor("o", [N, DIM], BF16, kind="ExternalOutput")

    with tile.TileContext(nc) as tc:
        for _ in range(unroll):
            _emit(nc, tc, x_d, cx_d, b1_d, b2_d, o_d, phase)
    nc.compile()
    return nc


def _emit(nc, tc, x_d, cx_d, b1_d, b2_d, o_d, phase=2):
    from contextlib import ExitStack

    ctx = ExitStack()
    with ctx:
        wpool = ctx.enter_context(tc.tile_pool(name="wpool", bufs=1))
        projp = ctx.enter_context(tc.tile_pool(name="projp", bufs=8))
        vnp = ctx.enter_context(tc.tile_pool(name="vnp", bufs=4))
        ppool = ctx.enter_context(tc.tile_pool(name="ppool", bufs=3))
        otp = ctx.enter_context(tc.tile_pool(name="otp", bufs=2))
        ostp = ctx.enter_context(tc.tile_pool(name="ostp", bufs=4))
        tiny = ctx.enter_context(tc.tile_pool(name="tiny", bufs=8))
        consts = ctx.enter_context(tc.tile_pool(name="consts", bufs=1))

        eps_col = consts.tile([128, 1], F32)
        nc.vector.memset(eps_col, EPS)
        ones_col2 = consts.tile([128, 8], BF16)
        nc.vector.memset(ones_col2, 1.0)

        # ---- input DMA stream. sync: b1, x-even, cxT transposes;
        # scalar: x-odd, win/wo. ----
        b1 = wpool.tile([128, B1_COLS], BF16, tag="b1")
        nc.sync.dma_start(out=b1, in_=b1_d[:])
        wcx = b1[:, B1_WCX:B1_WCX + 2048].rearrange("p (c k) -> p c k", k=256)
        identb = b1[:, B1_IDB:B1_IDB + 128]
        tri = b1[:, B1_TRI:B1_TRI + 128]
        pick3 = b1[0:3, B1_PICK:B1_PICK + 128]
        sel2 = b1[0:1, B1_SEL:B1_SEL + 256]

        natx = ctx.enter_context(tc.tile_pool(name="natx", bufs=1))
        xnat_t = natx.tile([128, NT, DIM], BF16, tag="nat", name="xnat")
        x_r = x_d.rearrange("(t p) d -> p t d", p=128)
        for hf in range(NT):
            eng = nc.sync if hf % 2 == 0 else nc.scalar
            eng.dma_start(out=xnat_t[:, hf:hf + 1, :], in_=x_r[:, hf:hf + 1, :])
        x_nat = [xnat_t[:, t, :] for t in range(NT)]

        # cxT via XBAR DMA transpose, chunk-major (single queue — see above)
        cxT_t = wpool.tile([128, KC, M], BF16, tag="cxT")
        for c in range(KC):
            nc.sync.dma_start(out=cxT_t[:, c, :],
                              in_=cx_d[:, c * 128:(c + 1) * 128],
                              transpose=True)

        b2 = wpool.tile([128, B2_COLS], BF16, tag="b2")
        nc.scalar.dma_start(out=b2[:, 0:B2_WK], in_=b2_d[:, 0:B2_WK])
        nc.scalar.dma_start(out=b2[:, B2_WK:], in_=b2_d[:, B2_WK:])
        winq = b2[:, B2_WQ:B2_WQ + 1152].rearrange("p (c k) -> p c k", k=128)
        wink = b2[:, B2_WK:B2_WK + 1152].rearrange("p (c k) -> p c k", k=128)
        winv = b2[:, B2_WV:B2_WV + 1152].rearrange("p (c k) -> p c k", k=128)
        wo = b2[:, B2_WO:B2_WO + 1024]

        # stat rows: row0 = -mu, row1 = std (aug contraction), row2 = rs
        srow = consts.tile([3, N], BF16)

        # per-token-half projection tiles: no false write-after-read deps
        # when the g1-half chains stream into attention-g0
        kcxT = projp.tile([128, M], BF16, tag="proj", name="kcxT")
        vcxT = projp.tile([128, M], BF16, tag="proj", name="vcxT")
        qTg = [projp.tile([128, 512], BF16, tag="proj", name=f"qT{g}")
               for g in range(2)]
        kinTg = [projp.tile([128, 512], BF16, tag="proj", name=f"kinT{g}")
                 for g in range(2)]
        vinTg = [projp.tile([128, 512], BF16, tag="proj", name=f"vinT{g}")
                 for g in range(2)]
        rsb = ctx.enter_context(tc.tile_pool(name="rsb", bufs=2))
        rs_bc = [rsb.tile([128, 512], F32, tag="rsbc", name=f"rsbc{g}")
                 for g in range(2)]
        vn = [None] * 16

        phase_a = ExitStack()
        with phase_a:
            tposed = phase_a.enter_context(tc.tile_pool(name="tposed", bufs=1))
            psA = phase_a.enter_context(
                tc.tile_pool(name="psA", bufs=1, space="PSUM"))

            # ---- x transposes on PE, one x-tile per PSUM batch so the
            # first batch only needs x tile 0; copies on ACT; bn_stats on
            # DVE per tile, post-processing batched at the end ----
            xT = tposed.tile([128, 2, KC, 512], BF16, tag="tp", name="xT")
            s4a = tiny.tile([128, NT, 4], F32, tag="s4a", name="s4a")
            for t in range(NT):
                ps = psA.tile([128, 1024], BF16, tag="tps", bufs=3,
                              name="tps")
                for c in range(KC):
                    nc.tensor.transpose(
                        ps[:, c * 128:(c + 1) * 128],
                        x_nat[t][:, c * 128:(c + 1) * 128], identb)
                co = (t % 4) * 128
                nc.scalar.copy(
                    out=xT[:, t // 4, :, co:co + 128],
                    in_=ps.rearrange("p (c k) -> p c k", k=128))
                bst = tiny.tile([128, 2, 6], F32, tag="bst", name="bst")
                for half in range(2):
                    nc.vector.bn_stats(
                        out=bst[:, half, :],
                        in_=x_nat[t][:, half * 512:(half + 1) * 512])
                nc.vector.bn_aggr(out=s4a[:, t, 0:2], in_=bst)
            # batched stats post-processing: cols 0:2 = (mu, var) per tile;
            # -> col 0 = -mu, col 1 = std, col 2 = rs
            nc.scalar.activation(
                out=s4a.rearrange("p t k -> p (t k)")[:, 1::4],
                in_=s4a.rearrange("p t k -> p (t k)")[:, 1::4],
                func=AF.Sqrt, bias=eps_col)
            nc.vector.reciprocal(
                out=s4a.rearrange("p t k -> p (t k)")[:, 2::4],
                in_=s4a.rearrange("p t k -> p (t k)")[:, 1::4])
            nc.vector.tensor_scalar(
                out=s4a.rearrange("p t k -> p (t k)")[:, 0::4],
                in0=s4a.rearrange("p t k -> p (t k)")[:, 0::4],
                scalar1=-1.0, scalar2=None, op0=ALU.mult)
            s4b = tiny.tile([128, NT, 3], BF16, tag="s4b", name="s4b")
            nc.vector.tensor_copy(out=s4b, in_=s4a[:, :, 0:3])

            # ---- context projections (cxT streamed by the DMA queue);
            # copies on ACT: DVE is running the stats chain ----
            for pj, dst in ((0, kcxT), (1, vcxT)):
                for gg in (0, 1):
                    sp = slice(gg * 512, (gg + 1) * 512)
                    ps = psA.tile([128, 512], F32, tag="pps", bufs=3,
                                  name="pps")
                    for c in range(KC):
                        nc.tensor.matmul(
                            ps, wcx[:, c, pj * 128:(pj + 1) * 128],
                            cxT_t[:, c, sp],
                            start=(c == 0), stop=(c == KC - 1))
                    nc.scalar.copy(out=dst[:, sp], in_=ps)

            # ---- stats rows (PE transposes are tiny; stats long done) ----
            for t in range(NT):
                ps2 = psA.tile([128, 512], BF16, tag="tpsr", bufs=2,
                               name="tpsr")
                nc.tensor.transpose(ps2[0:3, 0:128], s4b[:, t, :], identb)
                nc.vector.tensor_copy(
                    out=srow[:, t * 128:(t + 1) * 128], in_=ps2[0:3, 0:128])
            # rs broadcast tiles: pick3^T selects srow row 2 into every part
            for g in range(2):
                ps = psA.tile([128, 512], F32, tag="pps", bufs=3, name="pps")
                nc.tensor.matmul(
                    ps, pick3, srow[:, g * 512:(g + 1) * 512],
                    start=True, stop=True)
                nc.scalar.copy(out=rs_bc[g], in_=ps)

            # v_nat tiles: 4 j's per [128, 520] tile, each j = [64 vfeat h0 |
            # ones | 64 vfeat h1 | ones] so the PV stationary is contiguous.
            def v_transpose_half(src512, base):
                v_t = vnp.tile([128, 520], BF16, tag="vn", name=f"vn{base}")
                for jj in range(4):
                    vn[base + jj] = (v_t, jj)
                ps = psA.tile([128, 512], BF16, tag="tpsr", bufs=2,
                              name="tpsr")
                for jj in range(4):
                    nc.tensor.transpose(
                        ps[:, jj * 128:(jj + 1) * 128],
                        src512[:, jj * 128:(jj + 1) * 128], identb)
                nc.vector.tensor_copy(
                    out=v_t.rearrange("p (a b) -> p a b", b=65)[:, :, 64:65],
                    in_=ones_col2.rearrange("p (a b) -> p a b", b=1))
                nc.vector.tensor_copy(
                    out=v_t.rearrange("p (a b) -> p a b", b=65)[:, :, 0:64],
                    in_=ps.rearrange("p (a b) -> p a b", b=64))

            v_transpose_half(vcxT[:, 0:512], 0)
            v_transpose_half(vcxT[:, 512:1024], 4)

            # dummy exp: forces the Exp act-table load off the attention
            # start (the load costs ~1.3us on ACT)
            junk = tiny.tile([128, 1], BF16, tag="junk", name="junk")
            nc.scalar.activation(out=junk, in_=eps_col, func=AF.Exp)

            def in_chain(w9, dst, gg, pool, tag, bufs):
                """One input-projection half: 8 chunks + aug, rs on copy-out.
                Returns the matmul/copy thunks for interleaved emission."""
                st = {}
                sp = slice(gg * 512, (gg + 1) * 512)

                def step(c):
                    if c == 0:
                        st["ps"] = pool.tile([128, 512], F32, tag=tag,
                                             bufs=bufs, name=tag)
                    nc.tensor.matmul(
                        st["ps"], w9[:, c, :], xT[:, gg, c, :],
                        start=(c == 0), stop=False)

                def aug():
                    nc.tensor.matmul(
                        st["ps"], w9[0:2, KC, :], srow[0:2, sp],
                        start=False, stop=True)
                    nc.vector.tensor_tensor(
                        out=dst, in0=st["ps"], in1=rs_bc[gg], op=ALU.mult)

                return [lambda c=c: step(c) for c in range(KC)] + [aug]

            # g0-half projections (+ vin g1: its vn tiles are built here so
            # the attention scope needs no bf16 PSUM tag) run before
            # attention; q/kin g1 stream into attention-g0's exp-wait gaps.
            for w9, dst, gg in ((winq, qTg[0], 0), (wink, kinTg[0], 0),
                                (winv, vinTg[0], 0), (winv, vinTg[1], 1)):
                for th in in_chain(w9, dst, gg, psA, "pps", 3):
                    th()
            v_transpose_half(vinTg[0], 8)
            v_transpose_half(vinTg[1], 12)

            if phase == 1:
                for t, src_t in enumerate((qTg[0], kinTg[0], vinTg[0],
                                           kcxT[:, 0:512], vcxT[:, 0:512],
                                           qTg[1], kinTg[1], vinTg[1])):
                    nc.sync.dma_start(
                        out=o_d[t * 128:(t + 1) * 128, 0:512].bitcast(BF16),
                        in_=src_t)
                return

        # ---- attention + final projection ----
        with tc.tile_pool(name="psSim", bufs=1, space="PSUM") as psS, \
             tc.tile_pool(name="psO", bufs=1, space="PSUM") as psO, \
             tc.tile_pool(name="psF", bufs=1, space="PSUM") as psF:
            pend_final = [None]

            def final_head(g, o_ps):
                """lrec/lbc/oT chain. MUST be fully emitted before the next
                g's first PV (o_ps ring reuse is ordered by emission)."""
                lrec = [tiny.tile([1, 512], BF16, tag=f"lr{h}", bufs=2,
                                  name=f"lr{h}") for h in (0, 1)]
                with nc.allow_low_precision(reason="1/l in bf16 is plenty"):
                    for h in (0, 1):
                        nc.vector.tensor_copy(out=lrec[h],
                                              in_=o_ps[h][64:65, :])
                        nc.vector.reciprocal(out=lrec[h], in_=lrec[h])
                lbc_ps = psF.tile([128, 512], F32, tag="fin0", bufs=1,
                                  name="lbc")
                for h in (0, 1):
                    nc.tensor.matmul(
                        lbc_ps, sel2[:, 128 * h:128 * h + 128], lrec[h],
                        start=(h == 0), stop=(h == 1))
                lbc = tiny.tile([128, 512], F32, tag="lbc", bufs=2, name="lbc")
                nc.vector.tensor_copy(out=lbc, in_=lbc_ps)
                oT = otp.tile([128, 512], BF16, tag="oT")
                for h in (0, 1):
                    nc.vector.tensor_tensor(
                        out=oT[64 * h:64 * h + 64, :], in0=o_ps[h][0:64, :],
                        in1=lbc[64 * h:64 * h + 64, :], op=ALU.mult)
                pend_final[0] = None
                return oT

            def fin_tile(g, oT, t, tail):
                """Out-projection + store for one 128-token tile."""
                o_r = o_d.rearrange("(t p) d -> p t d", p=128)
                ost = ostp.tile([128, 1, DIM], BF16, tag="ost")
                for half in range(2):
                    wsp = slice(half * 512, (half + 1) * 512)
                    fp = psF.tile([128, 512], F32, tag=f"fin{half}",
                                  bufs=1, name=f"fin{half}")
                    nc.tensor.matmul(
                        fp, oT[:, t * 128:(t + 1) * 128], wo[:, wsp],
                        start=True, stop=True)
                    # at the tail ACT is idle (no more exp) — alternate
                    if tail and half == 1:
                        nc.scalar.copy(out=ost[:, 0, wsp], in_=fp)
                    else:
                        nc.vector.tensor_copy(out=ost[:, 0, wsp], in_=fp)
                eng = nc.sync if t % 2 == 0 else nc.scalar
                eng.dma_start(out=o_r[:, g * 4 + t:g * 4 + t + 1, :], in_=ost)

            # overlay work: q/kin g1 chains (PSUM: the idle fin tags) stream
            # into attention-g0's exp-wait gaps; final-g0's out-projection
            # tiles stream into attention-g1's.
            overlay = (in_chain(winq, qTg[1], 1, psF, "fin0", 1)
                       + in_chain(wink, kinTg[1], 1, psF, "fin1", 1))

            prev_g = [None]
            for g in (0, 1):
                # j order: cx0..cx6, in0.., cx7 (start/stop on full spans)
                j_list = [("cx", j) for j in range(7)]
                j_list += [("in", j) for j in range(4 * g + 4)]
                j_list.append(("cx", 7))
                n_j = len(j_list)
                o_ps = [psO.tile([128, 512], F32, tag=f"o{h}", name=f"ops{h}")
                        for h in (0, 1)]

                def j_meta(idx, g=g, j_list=j_list):
                    src, j = j_list[idx]
                    if src == "cx":
                        return kcxT, j, j, 0, False
                    off = max(0, 128 * (j - 4 * g))
                    return None, j, 8 + j, off, j >= 4 * g

                sims = [None] * n_j

                def emit_sim(idx, j_meta=j_meta, sims=sims, g=g):
                    kT, j, jg, off, diag = j_meta(idx)
                    ps = psS.tile([128, 1024], F32, tag="sim", bufs=2,
                                  name="sim")
                    for h in (0, 1):
                        hsl = slice(64 * h, 64 * h + 64)
                        if kT is None:
                            kop = kinTg[j // 4][hsl, (j % 4) * 128:
                                                (j % 4) * 128 + 128]
                        else:
                            kop = kT[hsl, j * 128:(j + 1) * 128]
                        nc.tensor.matmul(
                            ps[:, 512 * h + off:512 * (h + 1)],
                            kop, qTg[g][hsl, off:512],
                            start=True, stop=True)
                    sims[idx] = ps

                # software pipeline: sim for j+1 is emitted before PV of j so
                # the in-order PE computes the next sim while ACT runs exp.
                emit_sim(0)
                fin_steps = []
                if pend_final[0] is not None:
                    oT_prev = final_head(prev_g[0], pend_final[0])
                    fin_steps = [(prev_g[0], oT_prev, t) for t in range(4)]
                for idx in range(n_j):
                    if idx + 1 < n_j:
                        emit_sim(idx + 1)
                    if fin_steps and idx >= 2 and idx % 2 == 0:
                        pg, oTp, t = fin_steps.pop(0)
                        fin_tile(pg, oTp, t, tail=False)
                    for _ in range(2):
                        if overlay:
                            overlay.pop(0)()
                    kT, j, jg, off, diag = j_meta(idx)
                    p_t = ppool.tile([128, 1024], BF16, tag="p", name="p")
                    ps3 = sims[idx].rearrange("p (h t) -> p h t", h=2)
                    p3 = p_t.rearrange("p (h t) -> p h t", h=2)
                    nc.scalar.activation(
                        out=p3[:, :, off:512], in_=ps3[:, :, off:512],
                        func=AF.Exp)
                    if diag:
                        for h in (0, 1):
                            nc.gpsimd.tensor_tensor(
                                out=p_t[:, 512 * h + off:512 * h + off + 128],
                                in0=p_t[:, 512 * h + off:512 * h + off + 128],
                                in1=tri, op=ALU.mult)
                    sims[idx] = None
                    v_t, jj = vn[jg]
                    for h in (0, 1):
                        nc.tensor.matmul(
                            o_ps[h][0:65, off:512],
                            v_t[:, 130 * jj + 65 * h:130 * jj + 65 * h + 65],
                            p_t[:, 512 * h + off:512 * (h + 1)],
                            start=(idx == 0), stop=(idx == n_j - 1))
                for th in overlay:
                    th()
                overlay = []
                for pg, oTp, t in fin_steps:
                    fin_tile(pg, oTp, t, tail=False)
                pend_final[0] = o_ps
                prev_g[0] = g
            oT_last = final_head(1, pend_final[0])
            for t in range(4):
                fin_tile(1, oT_last, t, tail=True)


_NC_CACHE = None


def _get_nc():
    global _NC_CACHE
    if _NC_CACHE is None:
        _NC_CACHE = build_program()
    return _NC_CACHE


def make_in_maps(x, context, gamma, beta, Wq, Wkv, Wo, bo):
    import ml_dtypes
    BF = ml_dtypes.bfloat16
    x = np.asarray(x, np.float32)
    context = np.asarray(context, np.float32)
    gamma = np.asarray(gamma, np.float32)
    beta = np.asarray(beta, np.float32)
    Wq = np.asarray(Wq, np.float32)
    Wkv = np.asarray(Wkv, np.float32)
    Wo = np.asarray(Wo, np.float32)

    s = DH ** -0.5
    in_maps = []
    for core in range(8):
        b, hg = divmod(core, 4)
        cols = slice(128 * hg, 128 * hg + 128)
        wq = Wq[:, cols] * gamma[:, None] * s
        uq = wq.sum(0)
        bq = beta @ Wq[:, cols] * s
        wk = Wkv[:, :INNER][:, cols] * gamma[:, None]
        uk = wk.sum(0)
        bk = beta @ Wkv[:, :INNER][:, cols]
        wv = Wkv[:, INNER:][:, cols] * gamma[:, None]
        uv = wv.sum(0)
        bv = beta @ Wkv[:, INNER:][:, cols]

        # per-projection 9-chunk blocks (chunk 8 = aug rows u, b)
        def blk(w, u, bvec):
            out = np.zeros((128, KC + 1, 128), np.float32)
            for c in range(KC):
                out[:, c, :] = w[128 * c:128 * c + 128]
            out[0, KC, :] = u
            out[1, KC, :] = bvec
            return out.reshape(128, 1152)

        wcx = np.zeros((128, KC, 256), np.float32)
        for c in range(KC):
            rows = slice(128 * c, 128 * c + 128)
            wcx[:, c, 0:128] = Wkv[:, :INNER][rows, cols]
            wcx[:, c, 128:256] = Wkv[:, INNER:][rows, cols]

        b1 = np.zeros((128, B1_COLS), np.float32)
        b1[:, B1_WCX:B1_WCX + 2048] = wcx.reshape(128, 2048)
        b1[:, B1_IDB:B1_IDB + 128] = np.eye(128, dtype=np.float32)
        b1[:, B1_TRI:B1_TRI + 128] = np.tril(np.ones((128, 128), np.float32)).T
        b1[2, B1_PICK:B1_PICK + 128] = 1.0
        b1[0, B1_SEL:B1_SEL + 64] = 1.0
        b1[0, B1_SEL + 192:B1_SEL + 256] = 1.0

        b2 = np.zeros((128, B2_COLS), np.float32)
        b2[:, B2_WQ:B2_WQ + 1152] = blk(wq, uq, bq)
        b2[:, B2_WK:B2_WK + 1152] = blk(wk, uk, bk)
        b2[:, B2_WV:B2_WV + 1152] = blk(wv, uv, bv)
        b2[:, B2_WO:B2_WO + 1024] = Wo[cols, :]

        in_maps.append({
            "x": np.ascontiguousarray(x[b]).astype(BF),
            "cx": np.ascontiguousarray(context[b]).astype(BF),
            "b1": b1.astype(BF),
            "b2": b2.astype(BF),
        })
    return in_maps


def assemble(results, bo):
    bo = np.asarray(bo, np.float32)
    out = np.zeros((B, N, DIM), np.float32)
    for core in range(8):
        b = core // 4
        out[b] += results[core]["o"].astype(np.float32)
    out += bo[None, None, :]
    return out


def kernel(x, context, gamma, beta, Wq, Wkv, Wo, bo):
    nc = _get_nc()
    in_maps = make_in_maps(x, context, gamma, beta, Wq, Wkv, Wo, bo)
    res = run_bass_kernel_spmd(nc, in_maps, list(range(8)))
    return assemble(res.results, bo)


# revision 26
# speedup vs baseline: 1.4455x; 1.1330x over previous
"""Trainium2 Bass kernel for nn_CausalPrefixAttention (8-core SPMD), v3.1.

Changes vs v2 (119.6us):
  - cx is never loaded in natural layout: 8 XBAR DMA-transposes load cxT
    straight from HBM into SBUF, removing 64 PE transposes and 8 big
    PSUM->SBUF copies. ALL XBAR transposes share one queue: two concurrent
    XBAR DMAs on different queues corrupt each other (measured on device;
    per-16-token stripes of garbage). Regular DMAs on other queues are ok.
  - x still loads natural (bn_stats needs tokens-on-partitions); PE
    transposes it per-tile during the otherwise DMA-bound head (first PSUM
    batch needs only x tile 0), with all 8 PSUM->SBUF copies on ACT
    (idle then) and stats on DVE.
  - weights+consts packed into blob DMAs; win's q-block is a separate DMA
    so the q projection can start before the k/v blocks land.
  - sim PSUM is one [128,1024] f32 2-bank tile per j-tile (h0|h1), so exp
    is a single strided ACT instruction per j-tile instead of two.
  - causal tri-masking on gpsimd (Pool); out-projection PSUM->SBUF copies
    on DVE, keeping ACT = pure exp during attention.
  - final: both heads' 1/l in one reciprocal + one sel-matmul.
  - emission order (x-T, stats, cx-proj, q/k/v-proj, attention) matches
    DMA arrival so the in-order PE rarely stalls: the cost model halves PE
    clock for 3us after every stall.
"""

import os
import sys

for _p in ("/opt/trn_rl_repo", "/root/.axon_site/_ro/trn_rl_repo"):
    if os.path.isdir(_p) and _p not in sys.path:
        sys.path.append(_p)

import numpy as np

import concourse.mybir as mybir
import concourse.tile as tile
from concourse import bacc
from concourse.bass_utils import run_bass_kernel_spmd

F32 = mybir.dt.float32
BF16 = mybir.dt.bfloat16
AF = mybir.ActivationFunctionType
ALU = mybir.AluOpType

B, N, M, DIM, INNER, HEADS, DH = 2, 1024, 1024, 1024, 512, 8, 64
EPS = 1e-5
NT = N // 128      # token tiles per batch (8)
KC = DIM // 128    # contraction chunks (8)

# blob1 column offsets (bf16): wcx | idb | tri | pick | sel (row 0, 2x128)
B1_WCX, B1_IDB, B1_TRI, B1_PICK, B1_SEL = 0, 2048, 2176, 2304, 2432
B1_COLS = 2688
# blob2: win q-block | k-block | v-block | wo (split DMA: q early, rest later)
B2_WQ, B2_WK, B2_WV, B2_WO = 0, 1152, 2304, 3456
B2_COLS = 4480


def build_program(unroll=1, phase=2):
    nc = bacc.Bacc("TRN2", target_bir_lowering=False, debug=False)

    x_d = nc.dram_tensor("x", [N, DIM], BF16, kind="ExternalInput")
    cx_d = nc.dram_tensor("cx", [M, DIM], BF16, kind="ExternalInput")
    b1_d = nc.dram_tensor("b1", [128, B1_COLS], BF16, kind="ExternalInput")
    b2_d = nc.dram_tensor("b2", [128, B2_COLS], BF16, kind="ExternalInput")
    o_d = nc.dram_tensor("o", [N, DIM], BF16, kind="ExternalOutput")

    with tile.TileContext(nc) as tc:
        for _ in range(unroll):
            _emit(nc, tc, x_d, cx_d, b1_d, b2_d, o_d, phase)
    nc.compile()
    return nc


def _emit(nc, tc, x_d, cx_d, b1_d, b2_d, o_d, phase=2):
    from contextlib import ExitStack

    ctx = ExitStack()
    with ctx:
        wpool = ctx.enter_context(tc.tile_pool(name="wpool", bufs=1))
        projp = ctx.enter_context(tc.tile_pool(name="projp", bufs=8))
        vnp = ctx.enter_context(tc.tile_pool(name="vnp", bufs=4))
        ppool = ctx.enter_context(tc.tile_pool(name="ppool", bufs=3))
        otp = ctx.enter_context(tc.tile_pool(name="otp", bufs=2))
        ostp = ctx.enter_context(tc.tile_pool(name="ostp", bufs=4))
        tiny = ctx.enter_context(tc.tile_pool(name="tiny", bufs=8))
        consts = ctx.enter_context(tc.tile_pool(name="consts", bufs=1))

        eps_col = consts.tile([128, 1], F32)
        nc.vector.memset(eps_col, EPS)
        ones_col2 = consts.tile([128, 8], BF16)
        nc.vector.memset(ones_col2, 1.0)

        # ---- input DMA stream. DMA issue costs ~1.2us each on the HWDGE
        # queues (SEQ+HWDGE) and ACT-queue issues block ACT engine work, so:
        # sync queue = x pair-loads + the 8 XBAR transposes (consumption
        # order); Pool/SWDGE queue = all weight/const blobs (desc-gen runs
        # on the idle Pool engine, 25ns SEQ). Scalar issues nothing early.
        b1 = wpool.tile([128, B1_COLS], BF16, tag="b1")
        b2 = wpool.tile([128, B2_COLS], BF16, tag="b2")
        natx = ctx.enter_context(tc.tile_pool(name="natx", bufs=1))
        xnat_t = natx.tile([128, NT, DIM], BF16, tag="nat", name="xnat")
        x_r = x_d.rearrange("(t p) d -> p t d", p=128)
        for hp in range(NT // 2):
            nc.sync.dma_start(out=xnat_t[:, 2 * hp:2 * hp + 2, :],
                              in_=x_r[:, 2 * hp:2 * hp + 2, :])
        x_nat = [xnat_t[:, t, :] for t in range(NT)]

        # cxT via XBAR DMA transpose, chunk-major (single queue — see above)
        cxT_t = wpool.tile([128, KC, M], BF16, tag="cxT")
        for c in range(KC):
            nc.sync.dma_start(out=cxT_t[:, c, :],
                              in_=cx_d[:, c * 128:(c + 1) * 128],
                              transpose=True)

        nc.gpsimd.dma_start(out=b1[:, B1_IDB:], in_=b1_d[:, B1_IDB:])
        nc.gpsimd.dma_start(out=b2[:, 0:B2_WK], in_=b2_d[:, 0:B2_WK])
        nc.gpsimd.dma_start(out=b2[:, B2_WK:B2_WV], in_=b2_d[:, B2_WK:B2_WV])
        nc.gpsimd.dma_start(out=b1[:, 0:B1_IDB], in_=b1_d[:, 0:B1_IDB])
        nc.gpsimd.dma_start(out=b2[:, B2_WV:], in_=b2_d[:, B2_WV:])
        wcx = b1[:, B1_WCX:B1_WCX + 2048].rearrange("p (c k) -> p c k", k=256)
        identb = b1[:, B1_IDB:B1_IDB + 128]
        tri = b1[:, B1_TRI:B1_TRI + 128]
        pick3 = b1[0:3, B1_PICK:B1_PICK + 128]
        sel2 = b1[0:1, B1_SEL:B1_SEL + 256]
        winq = b2[:, B2_WQ:B2_WQ + 1152].rearrange("p (c k) -> p c k", k=128)
        wink = b2[:, B2_WK:B2_WK + 1152].rearrange("p (c k) -> p c k", k=128)
        winv = b2[:, B2_WV:B2_WV + 1152].rearrange("p (c k) -> p c k", k=128)
        wo = b2[:, B2_WO:B2_WO + 1024]
        # stat rows: row0 = -mu, row1 = std (aug contraction), row2 = rs
        srow = consts.tile([3, N], BF16)

        # per-token-half projection tiles: no false write-after-read deps
        # when the g1-half chains stream into attention-g0
        kcxT = projp.tile([128, M], BF16, tag="proj", name="kcxT")
        vcxT = projp.tile([128, M], BF16, tag="proj", name="vcxT")
        qTg = [projp.tile([128, 512], BF16, tag="proj", name=f"qT{g}")
               for g in range(2)]
        kinTg = [projp.tile([128, 512], BF16, tag="proj", name=f"kinT{g}")
                 for g in range(2)]
        vinTg = [projp.tile([128, 512], BF16, tag="proj", name=f"vinT{g}")
                 for g in range(2)]
        rsb = ctx.enter_context(tc.tile_pool(name="rsb", bufs=2))
        rs_bc = [rsb.tile([128, 512], F32, tag="rsbc", name=f"rsbc{g}")
                 for g in range(2)]
        vn = [None] * 16

        phase_a = ExitStack()
        with phase_a:
            tposed = phase_a.enter_context(tc.tile_pool(name="tposed", bufs=1))
            psA = phase_a.enter_context(
                tc.tile_pool(name="psA", bufs=1, space="PSUM"))

            # ---- x transposes on PE, one x-tile per PSUM batch so the
            # first batch only needs x tile 0; copies on ACT; bn_stats on
            # DVE per tile, post-processing batched at the end ----
            xT = tposed.tile([128, 2, KC, 512], BF16, tag="tp", name="xT")
            s4a = tiny.tile([128, NT, 4], F32, tag="s4a", name="s4a")
            for t in range(NT):
                ps = psA.tile([128, 1024], BF16, tag="tps", bufs=2,
                              name="tps")
                for c in range(KC):
                    nc.tensor.transpose(
                        ps[:, c * 128:(c + 1) * 128],
                        x_nat[t][:, c * 128:(c + 1) * 128], identb)
                co = (t % 4) * 128
                nc.scalar.copy(
                    out=xT[:, t // 4, :, co:co + 128],
                    in_=ps.rearrange("p (c k) -> p c k", k=128))
                bst = tiny.tile([128, 2, 6], F32, tag="bst", name="bst")
                for half in range(2):
                    nc.vector.bn_stats(
                        out=bst[:, half, :],
                        in_=x_nat[t][:, half * 512:(half + 1) * 512])
                nc.vector.bn_aggr(out=s4a[:, t, 0:2], in_=bst)

            # batched stats post-processing: cols 0:2 = (mu, var) per tile;
            # -> col 0 = -mu, col 1 = std, col 2 = rs
            nc.scalar.activation(
                out=s4a.rearrange("p t k -> p (t k)")[:, 1::4],
                in_=s4a.rearrange("p t k -> p (t k)")[:, 1::4],
                func=AF.Sqrt, bias=eps_col)
            nc.vector.reciprocal(
                out=s4a.rearrange("p t k -> p (t k)")[:, 2::4],
                in_=s4a.rearrange("p t k -> p (t k)")[:, 1::4])
            nc.vector.tensor_scalar(
                out=s4a.rearrange("p t k -> p (t k)")[:, 0::4],
                in0=s4a.rearrange("p t k -> p (t k)")[:, 0::4],
                scalar1=-1.0, scalar2=None, op0=ALU.mult)
            s4b = tiny.tile([128, NT, 3], BF16, tag="s4b", name="s4b")
            nc.vector.tensor_copy(out=s4b, in_=s4a[:, :, 0:3])

            def in_chain(w9, dst, gg, pool, tag, bufs):
                """One input-projection half: 8 chunks + aug, rs on copy-out.
                Returns the matmul/copy thunks for interleaved emission."""
                st = {}
                sp = slice(gg * 512, (gg + 1) * 512)

                def step(c):
                    if c == 0:
                        st["ps"] = pool.tile([128, 512], F32, tag=tag,
                                             bufs=bufs, name=tag)
                    nc.tensor.matmul(
                        st["ps"], w9[:, c, :], xT[:, gg, c, :],
                        start=(c == 0), stop=False)

                def aug():
                    nc.tensor.matmul(
                        st["ps"], w9[0:2, KC, :], srow[0:2, sp],
                        start=False, stop=True)
                    nc.vector.tensor_tensor(
                        out=dst, in0=st["ps"], in1=rs_bc[gg], op=ALU.mult)

                return [lambda c=c: step(c) for c in range(KC)] + [aug]

            # projections run in pairs around the stats-row build so the
            # 4-tag PSUM ring always has 2 free banks for rs_bc/next pair
            pair_a = [in_chain(winq, qTg[0], 0, psA, "pps0", 1),
                      in_chain(wink, kinTg[0], 0, psA, "pps1", 1)]
            for ch in pair_a:
                for th in ch[:-1]:
                    th()

            # ---- stats rows (PE transposes are tiny; stats long done) ----
            for t in range(NT):
                ps2 = psA.tile([128, 512], BF16, tag="tpsr", bufs=2,
                               name="tpsr")
                nc.tensor.transpose(ps2[0:3, 0:128], s4b[:, t, :], identb)
                nc.vector.tensor_copy(
                    out=srow[:, t * 128:(t + 1) * 128], in_=ps2[0:3, 0:128])
            # rs broadcast tiles: pick3^T selects srow row 2 into every part
            for g in range(2):
                ps = psA.tile([128, 512], F32, tag=f"pps{2 + g}", bufs=1,
                              name=f"pps{2 + g}")
                nc.tensor.matmul(
                    ps, pick3, srow[:, g * 512:(g + 1) * 512],
                    start=True, stop=True)
                nc.scalar.copy(out=rs_bc[g], in_=ps)
            for ch in pair_a:
                ch[-1]()

            pair_b = [in_chain(winv, vinTg[0], 0, psA, "pps0", 1),
                      in_chain(winv, vinTg[1], 1, psA, "pps1", 1)]
            for ch in pair_b:
                for th in ch:
                    th()

            # ---- context projections (cxT streamed by the DMA queue);
            # copies on ACT ----
            ci = 0
            for pj, dst in ((0, kcxT), (1, vcxT)):
                for gg in (0, 1):
                    sp = slice(gg * 512, (gg + 1) * 512)
                    ps = psA.tile([128, 512], F32, tag=f"pps{(2 + ci) % 4}",
                                  bufs=1, name=f"pps{(2 + ci) % 4}")
                    ci += 1
                    for c in range(KC):
                        nc.tensor.matmul(
                            ps, wcx[:, c, pj * 128:(pj + 1) * 128],
                            cxT_t[:, c, sp],
                            start=(c == 0), stop=(c == KC - 1))
                    nc.scalar.copy(out=dst[:, sp], in_=ps)

            # v_nat tiles: 4 j's per [128, 520] tile, each j = [64 vfeat h0 |
            # ones | 64 vfeat h1 | ones] so the PV stationary is contiguous.
            def v_transpose_half(src512, base):
                v_t = vnp.tile([128, 520], BF16, tag="vn", name=f"vn{base}")
                for jj in range(4):
                    vn[base + jj] = (v_t, jj)
                ps = psA.tile([128, 512], BF16, tag="tpsr", bufs=2,
                              name="tpsr")
                for jj in range(4):
                    nc.tensor.transpose(
                        ps[:, jj * 128:(jj + 1) * 128],
                        src512[:, jj * 128:(jj + 1) * 128], identb)
                nc.vector.tensor_copy(
                    out=v_t.rearrange("p (a b) -> p a b", b=65)[:, :, 64:65],
                    in_=ones_col2.rearrange("p (a b) -> p a b", b=1))
                nc.vector.tensor_copy(
                    out=v_t.rearrange("p (a b) -> p a b", b=65)[:, :, 0:64],
                    in_=ps.rearrange("p (a b) -> p a b", b=64))

            v_transpose_half(vinTg[0], 8)
            v_transpose_half(vinTg[1], 12)
            v_transpose_half(vcxT[:, 0:512], 0)
            v_transpose_half(vcxT[:, 512:1024], 4)

            # dummy exp: forces the Exp act-table load off the attention
            # start (the load costs ~1.3us on ACT)
            junk = tiny.tile([128, 1], BF16, tag="junk", name="junk")
            nc.scalar.activation(out=junk, in_=eps_col, func=AF.Exp)

            if phase == 1:
                for t, src_t in enumerate((qTg[0], kinTg[0], vinTg[0],
                                           kcxT[:, 0:512], vcxT[:, 0:512],
                                           qTg[1], kinTg[1], vinTg[1])):
                    nc.sync.dma_start(
                        out=o_d[t * 128:(t + 1) * 128, 0:512].bitcast(BF16),
                        in_=src_t)
                return

        # ---- attention + final projection ----
        with tc.tile_pool(name="psSim", bufs=1, space="PSUM") as psS, \
             tc.tile_pool(name="psO", bufs=1, space="PSUM") as psO, \
             tc.tile_pool(name="psF", bufs=1, space="PSUM") as psF:
            pend_final = [None]

            def final_head(g, o_ps):
                """lrec/lbc/oT chain. MUST be fully emitted before the next
                g's first PV (o_ps ring reuse is ordered by emission)."""
                lrec = [tiny.tile([1, 512], BF16, tag=f"lr{h}", bufs=2,
                                  name=f"lr{h}") for h in (0, 1)]
                with nc.allow_low_precision(reason="1/l in bf16 is plenty"):
                    for h in (0, 1):
                        nc.vector.tensor_copy(out=lrec[h],
                                              in_=o_ps[h][64:65, :])
                        nc.vector.reciprocal(out=lrec[h], in_=lrec[h])
                lbc_ps = psF.tile([128, 512], F32, tag="fin0", bufs=1,
                                  name="lbc")
                for h in (0, 1):
                    nc.tensor.matmul(
                        lbc_ps, sel2[:, 128 * h:128 * h + 128], lrec[h],
                        start=(h == 0), stop=(h == 1))
                lbc = tiny.tile([128, 512], F32, tag="lbc", bufs=2, name="lbc")
                nc.vector.tensor_copy(out=lbc, in_=lbc_ps)
                oT = otp.tile([128, 512], BF16, tag="oT")
                for h in (0, 1):
                    nc.vector.tensor_tensor(
                        out=oT[64 * h:64 * h + 64, :], in0=o_ps[h][0:64, :],
                        in1=lbc[64 * h:64 * h + 64, :], op=ALU.mult)
                pend_final[0] = None
                return oT

            def fin_tile(g, oT, t, tail):
                """Out-projection + store for one 128-token tile."""
                o_r = o_d.rearrange("(t p) d -> p t d", p=128)
                ost = ostp.tile([128, 1, DIM], BF16, tag="ost")
                for half in range(2):
                    wsp = slice(half * 512, (half + 1) * 512)
                    fp = psF.tile([128, 512], F32, tag=f"fin{half}",
                                  bufs=1, name=f"fin{half}")
                    nc.tensor.matmul(
                        fp, oT[:, t * 128:(t + 1) * 128], wo[:, wsp],
                        start=True, stop=True)
                    # at the tail ACT is idle (no more exp) — alternate
                    if tail and half == 1:
                        nc.scalar.copy(out=ost[:, 0, wsp], in_=fp)
                    else:
                        nc.vector.tensor_copy(out=ost[:, 0, wsp], in_=fp)
                eng = nc.sync if t % 2 == 0 else nc.scalar
                eng.dma_start(out=o_r[:, g * 4 + t:g * 4 + t + 1, :], in_=ost)

            # overlay work: q/kin g1 chains (PSUM: the idle fin tags) stream
            # into attention-g0's exp-wait gaps; final-g0's out-projection
            # tiles stream into attention-g1's.
            overlay = (in_chain(winq, qTg[1], 1, psF, "fin0", 1)
                       + in_chain(wink, kinTg[1], 1, psF, "fin1", 1))

            prev_g = [None]
            for g in (0, 1):
                # j order: cx0..cx6, in0.., cx7 (start/stop on full spans)
                j_list = [("cx", j) for j in range(7)]
                j_list += [("in", j) for j in range(4 * g + 4)]
                j_list.append(("cx", 7))
                n_j = len(j_list)
                o_ps = [psO.tile([128, 512], F32, tag=f"o{h}", name=f"ops{h}")
                        for h in (0, 1)]

                def j_meta(idx, g=g, j_list=j_list):
                    src, j = j_list[idx]
                    if src == "cx":
                        return kcxT, j, j, 0, False
                    off = max(0, 128 * (j - 4 * g))
                    return None, j, 8 + j, off, j >= 4 * g

                sims = [None] * n_j

                def emit_sim(idx, j_meta=j_meta, sims=sims, g=g):
                    kT, j, jg, off, diag = j_meta(idx)
                    ps = psS.tile([128, 1024], F32, tag="sim", bufs=2,
                                  name="sim")
                    for h in (0, 1):
                        hsl = slice(64 * h, 64 * h + 64)
                        if kT is None:
                            kop = kinTg[j // 4][hsl, (j % 4) * 128:
                                                (j % 4) * 128 + 128]
                        else:
                            kop = kT[hsl, j * 128:(j + 1) * 128]
                        nc.tensor.matmul(
                            ps[:, 512 * h + off:512 * (h + 1)],
                            kop, qTg[g][hsl, off:512],
                            start=True, stop=True)
                    sims[idx] = ps

                # software pipeline: sim for j+1 is emitted before PV of j so
                # the in-order PE computes the next sim while ACT runs exp.
                emit_sim(0)
                fin_steps = []
                if pend_final[0] is not None:
                    oT_prev = final_head(prev_g[0], pend_final[0])
                    fin_steps = [(prev_g[0], oT_prev, t) for t in range(4)]
                for idx in range(n_j):
                    if idx + 1 < n_j:
                        emit_sim(idx + 1)
                    if fin_steps and idx >= 2 and idx % 2 == 0:
                        pg, oTp, t = fin_steps.pop(0)
                        fin_tile(pg, oTp, t, tail=False)
                    for _ in range(2):
                        if overlay:
                            overlay.pop(0)()
                    kT, j, jg, off, diag = j_meta(idx)
                    p_t = ppool.tile([128, 1024], BF16, tag="p", name="p")
                    ps3 = sims[idx].rearrange("p (h t) -> p h t", h=2)
                    p3 = p_t.rearrange("p (h t) -> p h t", h=2)
                    nc.scalar.activation(
                        out=p3[:, :, off:512], in_=ps3[:, :, off:512],
                        func=AF.Exp)
                    if diag:
                        for h in (0, 1):
                            nc.gpsimd.tensor_tensor(
                                out=p_t[:, 512 * h + off:512 * h + off + 128],
                                in0=p_t[:, 512 * h + off:512 * h + off + 128],
                                in1=tri, op=ALU.mult)
                    sims[idx] = None
                    v_t, jj = vn[jg]
                    for h in (0, 1):
                        nc.tensor.matmul(
                            o_ps[h][0:65, off:512],
                            v_t[:, 130 * jj + 65 * h:130 * jj + 65 * h + 65],
                            p_t[:, 512 * h + off:512 * (h + 1)],
                            start=(idx == 0), stop=(idx == n_j - 1))
                for th in overlay:
                    th()
                overlay = []
                for pg, oTp, t in fin_steps:
                    fin_tile(pg, oTp, t, tail=False)
                pend_final[0] = o_ps
                prev_g[0] = g
            oT_last = final_head(1, pend_final[0])
            for t in range(4):
                fin_tile(1, oT_last, t, tail=True)


_NC_CACHE = None


def _get_nc():
    global _NC_CACHE
    if _NC_CACHE is None:
        _NC_CACHE = build_program()
    return _NC_CACHE


def make_in_maps(x, context, gamma, beta, Wq, Wkv, Wo, bo):
    import ml_dtypes
    BF = ml_dtypes.bfloat16
    x = np.asarray(x, np.float32)
    context = np.asarray(context, np.float32)
    gamma = np.asarray(gamma, np.float32)
    beta = np.asarray(beta, np.float32)
    Wq = np.asarray(Wq, np.float32)
    Wkv = np.asarray(Wkv, np.float32)
    Wo = np.asarray(Wo, np.float32)

    s = DH ** -0.5
    in_maps = []
    for core in range(8):
        b, hg = divmod(core, 4)
        cols = slice(128 * hg, 128 * hg + 128)
        wq = Wq[:, cols] * gamma[:, None] * s
        uq = wq.sum(0)
        bq = beta @ Wq[:, cols] * s
        wk = Wkv[:, :INNER][:, cols] * gamma[:, None]
        uk = wk.sum(0)
        bk = beta @ Wkv[:, :INNER][:, cols]
        wv = Wkv[:, INNER:][:, cols] * gamma[:, None]
        uv = wv.sum(0)
        bv = beta @ Wkv[:, INNER:][:, cols]

        # per-projection 9-chunk blocks (chunk 8 = aug rows u, b)
        def blk(w, u, bvec):
            out = np.zeros((128, KC + 1, 128), np.float32)
            for c in range(KC):
                out[:, c, :] = w[128 * c:128 * c + 128]
            out[0, KC, :] = u
            out[1, KC, :] = bvec
            return out.reshape(128, 1152)

        wcx = np.zeros((128, KC, 256), np.float32)
        for c in range(KC):
            rows = slice(128 * c, 128 * c + 128)
            wcx[:, c, 0:128] = Wkv[:, :INNER][rows, cols]
            wcx[:, c, 128:256] = Wkv[:, INNER:][rows, cols]

        b1 = np.zeros((128, B1_COLS), np.float32)
        b1[:, B1_WCX:B1_WCX + 2048] = wcx.reshape(128, 2048)
        b1[:, B1_IDB:B1_IDB + 128] = np.eye(128, dtype=np.float32)
        b1[:, B1_TRI:B1_TRI + 128] = np.tril(np.ones((128, 128), np.float32)).T
        b1[2, B1_PICK:B1_PICK + 128] = 1.0
        b1[0, B1_SEL:B1_SEL + 64] = 1.0
        b1[0, B1_SEL + 192:B1_SEL + 256] = 1.0

        b2 = np.zeros((128, B2_COLS), np.float32)
        b2[:, B2_WQ:B2_WQ + 1152] = blk(wq, uq, bq)
        b2[:, B2_WK:B2_WK + 1152] = blk(wk, uk, bk)
        b2[:, B2_WV:B2_WV + 1152] = blk(wv, uv, bv)
        b2[:, B2_WO:B2_WO + 1024] = Wo[cols, :]

        in_maps.append({
            "x": np.ascontiguousarray(x[b]).astype(BF),
            "cx": np.ascontiguousarray(context[b]).astype(BF),
            "b1": b1.astype(BF),
            "b2": b2.astype(BF),
        })
    return in_maps


def assemble(results, bo):
    bo = np.asarray(bo, np.float32)
    out = np.zeros((B, N, DIM), np.float32)
    for core in range(8):
        b = core // 4
        out[b] += results[core]["o"].astype(np.float32)
    out += bo[None, None, :]
    return out


def kernel(x, context, gamma, beta, Wq, Wkv, Wo, bo):
    nc = _get_nc()
    in_maps = make_in_maps(x, context, gamma, beta, Wq, Wkv, Wo, bo)
    res = run_bass_kernel_spmd(nc, in_maps, list(range(8)))
    return assemble(res.results, bo)


# revision 35
# speedup vs baseline: 1.5106x; 1.0451x over previous
"""Trainium2 Bass kernel for nn_CausalPrefixAttention (8-core SPMD), v3.1.

Changes vs v2 (119.6us):
  - cx is never loaded in natural layout: 8 XBAR DMA-transposes load cxT
    straight from HBM into SBUF, removing 64 PE transposes and 8 big
    PSUM->SBUF copies. ALL XBAR transposes share one queue: two concurrent
    XBAR DMAs on different queues corrupt each other (measured on device;
    per-16-token stripes of garbage). Regular DMAs on other queues are ok.
  - x still loads natural (bn_stats needs tokens-on-partitions); PE
    transposes it per-tile during the otherwise DMA-bound head (first PSUM
    batch needs only x tile 0), with all 8 PSUM->SBUF copies on ACT
    (idle then) and stats on DVE.
  - weights+consts packed into blob DMAs; win's q-block is a separate DMA
    so the q projection can start before the k/v blocks land.
  - sim PSUM is one [128,1024] f32 2-bank tile per j-tile (h0|h1), so exp
    is a single strided ACT instruction per j-tile instead of two.
  - causal tri-masking on gpsimd (Pool); out-projection PSUM->SBUF copies
    on DVE, keeping ACT = pure exp during attention.
  - final: both heads' 1/l in one reciprocal + one sel-matmul.
  - emission order (x-T, stats, cx-proj, q/k/v-proj, attention) matches
    DMA arrival so the in-order PE rarely stalls: the cost model halves PE
    clock for 3us after every stall.
"""

import os
import sys

for _p in ("/opt/trn_rl_repo", "/root/.axon_site/_ro/trn_rl_repo"):
    if os.path.isdir(_p) and _p not in sys.path:
        sys.path.append(_p)

import numpy as np

import concourse.mybir as mybir
import concourse.tile as tile
from concourse import bacc
from concourse.bass_utils import run_bass_kernel_spmd

F32 = mybir.dt.float32
BF16 = mybir.dt.bfloat16
AF = mybir.ActivationFunctionType
ALU = mybir.AluOpType

B, N, M, DIM, INNER, HEADS, DH = 2, 1024, 1024, 1024, 512, 8, 64
EPS = 1e-5
NT = N // 128      # token tiles per batch (8)
KC = DIM // 128    # contraction chunks (8)

# blob1 column offsets (bf16): wcx | idb | tri | pick | sel (row 0, 2x128)
B1_WCX, B1_IDB, B1_TRI, B1_PICK, B1_SEL = 0, 2048, 2176, 2304, 2432
B1_COLS = 2688
# blob2: win q-block | k-block | v-block | wo (split DMA: q early, rest later)
B2_WQ, B2_WK, B2_WV, B2_WO = 0, 1152, 2304, 3456
B2_COLS = 4480


def build_program(unroll=1, phase=2):
    nc = bacc.Bacc("TRN2", target_bir_lowering=False, debug=False)

    x_d = nc.dram_tensor("x", [N, DIM], BF16, kind="ExternalInput")
    cx_d = nc.dram_tensor("cx", [M, DIM], BF16, kind="ExternalInput")
    b1_d = nc.dram_tensor("b1", [128, B1_COLS], BF16, kind="ExternalInput")
    b2_d = nc.dram_tensor("b2", [128, B2_COLS], BF16, kind="ExternalInput")
    o_d = nc.dram_tensor("o", [N, DIM], BF16, kind="ExternalOutput")

    with tile.TileContext(nc) as tc:
        for _ in range(unroll):
            _emit(nc, tc, x_d, cx_d, b1_d, b2_d, o_d, phase)
    nc.compile()
    return nc


def _emit(nc, tc, x_d, cx_d, b1_d, b2_d, o_d, phase=2):
    from contextlib import ExitStack

    ctx = ExitStack()
    with ctx:
        wpool = ctx.enter_context(tc.tile_pool(name="wpool", bufs=1))
        projp = ctx.enter_context(tc.tile_pool(name="projp", bufs=8))
        vnp = ctx.enter_context(tc.tile_pool(name="vnp", bufs=4))
        ppool = ctx.enter_context(tc.tile_pool(name="ppool", bufs=3))
        otp = ctx.enter_context(tc.tile_pool(name="otp", bufs=2))
        ostp = ctx.enter_context(tc.tile_pool(name="ostp", bufs=4))
        tiny = ctx.enter_context(tc.tile_pool(name="tiny", bufs=8))
        consts = ctx.enter_context(tc.tile_pool(name="consts", bufs=1))

        eps_col = consts.tile([128, 1], F32)
        nc.vector.memset(eps_col, EPS)
        ones_col2 = consts.tile([128, 8], BF16)
        nc.vector.memset(ones_col2, 1.0)

        # ---- input DMA stream. DMA issue costs ~1.2us each on the HWDGE
        # queues (SEQ+HWDGE) and ACT-queue issues block ACT engine work, so:
        # sync queue = x pair-loads + the 8 XBAR transposes (consumption
        # order); Pool/SWDGE queue = all weight/const blobs (desc-gen runs
        # on the idle Pool engine, 25ns SEQ). Scalar issues nothing early.
        b1 = wpool.tile([128, B1_COLS], BF16, tag="b1")
        b2 = wpool.tile([128, B2_COLS], BF16, tag="b2")
        natx = ctx.enter_context(tc.tile_pool(name="natx", bufs=1))
        xnat_t = natx.tile([128, NT, DIM], BF16, tag="nat", name="xnat")
        x_r = x_d.rearrange("(t p) d -> p t d", p=128)
        for hp in range(NT // 2):
            nc.sync.dma_start(out=xnat_t[:, 2 * hp:2 * hp + 2, :],
                              in_=x_r[:, 2 * hp:2 * hp + 2, :])
        x_nat = [xnat_t[:, t, :] for t in range(NT)]

        # cxT via XBAR DMA transpose, chunk-major (single queue — see above)
        cxT_t = wpool.tile([128, KC, M], BF16, tag="cxT")
        for c in range(KC):
            nc.sync.dma_start(out=cxT_t[:, c, :],
                              in_=cx_d[:, c * 128:(c + 1) * 128],
                              transpose=True)

        nc.gpsimd.dma_start(out=b1[:, B1_IDB:], in_=b1_d[:, B1_IDB:])
        nc.gpsimd.dma_start(out=b2[:, 0:B2_WK], in_=b2_d[:, 0:B2_WK])
        nc.gpsimd.dma_start(out=b2[:, B2_WK:B2_WV], in_=b2_d[:, B2_WK:B2_WV])
        nc.gpsimd.dma_start(out=b1[:, 0:B1_IDB], in_=b1_d[:, 0:B1_IDB])
        nc.gpsimd.dma_start(out=b2[:, B2_WV:], in_=b2_d[:, B2_WV:])
        wcx = b1[:, B1_WCX:B1_WCX + 2048].rearrange("p (c k) -> p c k", k=256)
        identb = b1[:, B1_IDB:B1_IDB + 128]
        tri = b1[:, B1_TRI:B1_TRI + 128]
        pick3 = b1[0:3, B1_PICK:B1_PICK + 128]
        sel2 = b1[0:1, B1_SEL:B1_SEL + 256]
        winq = b2[:, B2_WQ:B2_WQ + 1152].rearrange("p (c k) -> p c k", k=128)
        wink = b2[:, B2_WK:B2_WK + 1152].rearrange("p (c k) -> p c k", k=128)
        winv = b2[:, B2_WV:B2_WV + 1152].rearrange("p (c k) -> p c k", k=128)
        wo = b2[:, B2_WO:B2_WO + 1024]
        # stat rows: row0 = -mu, row1 = std (aug contraction), row2 = rs
        srow = consts.tile([3, N], BF16)

        # per-token-half projection tiles: no false write-after-read deps
        # when the g1-half chains stream into attention-g0
        kcxT = projp.tile([128, M], BF16, tag="proj", name="kcxT")
        vcxT = projp.tile([128, M], BF16, tag="proj", name="vcxT")
        qTg = [projp.tile([128, 512], BF16, tag="proj", name=f"qT{g}")
               for g in range(2)]
        kinTg = [projp.tile([128, 512], BF16, tag="proj", name=f"kinT{g}")
                 for g in range(2)]
        vinTg = [projp.tile([128, 512], BF16, tag="proj", name=f"vinT{g}")
                 for g in range(2)]
        rsb = ctx.enter_context(tc.tile_pool(name="rsb", bufs=2))
        rs_bc = [rsb.tile([128, 512], F32, tag="rsbc", name=f"rsbc{g}")
                 for g in range(2)]
        vn = [None] * 16

        phase_a = ExitStack()
        with phase_a:
            tposed = phase_a.enter_context(tc.tile_pool(name="tposed", bufs=1))
            psA = phase_a.enter_context(
                tc.tile_pool(name="psA", bufs=1, space="PSUM"))

            # ---- x transposes on PE, one x-tile per PSUM batch so the
            # first batch only needs x tile 0; copies on ACT; bn_stats on
            # DVE per tile, post-processing batched at the end ----
            xT = tposed.tile([128, 2, KC, 512], BF16, tag="tp", name="xT")
            s4a = tiny.tile([128, NT, 4], F32, tag="s4a", name="s4a")
            for t in range(NT):
                ps = psA.tile([128, 1024], BF16, tag="tps", bufs=2,
                              name="tps")
                for c in range(KC):
                    nc.tensor.transpose(
                        ps[:, c * 128:(c + 1) * 128],
                        x_nat[t][:, c * 128:(c + 1) * 128], identb)
                co = (t % 4) * 128
                nc.scalar.copy(
                    out=xT[:, t // 4, :, co:co + 128],
                    in_=ps.rearrange("p (c k) -> p c k", k=128))
                bst = tiny.tile([128, 2, 6], F32, tag="bst", name="bst")
                for half in range(2):
                    nc.vector.bn_stats(
                        out=bst[:, half, :],
                        in_=x_nat[t][:, half * 512:(half + 1) * 512])
                nc.vector.bn_aggr(out=s4a[:, t, 0:2], in_=bst)

            # batched stats post-processing: cols 0:2 = (mu, var) per tile;
            # -> col 0 = -mu, col 1 = std, col 2 = rs
            nc.scalar.activation(
                out=s4a.rearrange("p t k -> p (t k)")[:, 1::4],
                in_=s4a.rearrange("p t k -> p (t k)")[:, 1::4],
                func=AF.Sqrt, bias=eps_col)
            nc.vector.reciprocal(
                out=s4a.rearrange("p t k -> p (t k)")[:, 2::4],
                in_=s4a.rearrange("p t k -> p (t k)")[:, 1::4])
            nc.vector.tensor_scalar(
                out=s4a.rearrange("p t k -> p (t k)")[:, 0::4],
                in0=s4a.rearrange("p t k -> p (t k)")[:, 0::4],
                scalar1=-1.0, scalar2=None, op0=ALU.mult)
            s4b = tiny.tile([128, NT, 3], BF16, tag="s4b", name="s4b")
            nc.vector.tensor_copy(out=s4b, in_=s4a[:, :, 0:3])

            def in_chain(w9, dst, gg, pool, tag, bufs):
                """One input-projection half: 8 chunks + aug, rs on copy-out.
                Returns the matmul/copy thunks for interleaved emission."""
                st = {}
                sp = slice(gg * 512, (gg + 1) * 512)

                def step(c):
                    if c == 0:
                        st["ps"] = pool.tile([128, 512], F32, tag=tag,
                                             bufs=bufs, name=tag)
                    nc.tensor.matmul(
                        st["ps"], w9[:, c, :], xT[:, gg, c, :],
                        start=(c == 0), stop=False)

                def aug():
                    nc.tensor.matmul(
                        st["ps"], w9[0:2, KC, :], srow[0:2, sp],
                        start=False, stop=True)
                    nc.vector.tensor_tensor(
                        out=dst, in0=st["ps"], in1=rs_bc[gg], op=ALU.mult)

                return [lambda c=c: step(c) for c in range(KC)] + [aug]

            # projections run in pairs around the stats-row build so the
            # 4-tag PSUM ring always has 2 free banks for rs_bc/next pair
            pair_a = [in_chain(winq, qTg[0], 0, psA, "pps0", 1),
                      in_chain(wink, kinTg[0], 0, psA, "pps1", 1)]
            for ch in pair_a:
                for th in ch[:-1]:
                    th()

            # ---- stats rows (PE transposes are tiny; stats long done) ----
            for t in range(NT):
                ps2 = psA.tile([128, 512], BF16, tag="tpsr", bufs=2,
                               name="tpsr")
                nc.tensor.transpose(ps2[0:3, 0:128], s4b[:, t, :], identb)
                nc.vector.tensor_copy(
                    out=srow[:, t * 128:(t + 1) * 128], in_=ps2[0:3, 0:128])
            # rs broadcast tiles: pick3^T selects srow row 2 into every part
            for g in range(2):
                ps = psA.tile([128, 512], F32, tag=f"pps{2 + g}", bufs=1,
                              name=f"pps{2 + g}")
                nc.tensor.matmul(
                    ps, pick3, srow[:, g * 512:(g + 1) * 512],
                    start=True, stop=True)
                nc.scalar.copy(out=rs_bc[g], in_=ps)
            for ch in pair_a:
                ch[-1]()

            pair_b = [in_chain(winv, vinTg[0], 0, psA, "pps0", 1),
                      in_chain(winv, vinTg[1], 1, psA, "pps1", 1)]
            for ch in pair_b:
                for th in ch:
                    th()

            # ---- context projections (cxT streamed by the DMA queue);
            # copies on ACT ----
            ci = 0
            for pj, dst in ((0, kcxT), (1, vcxT)):
                for gg in (0, 1):
                    sp = slice(gg * 512, (gg + 1) * 512)
                    ps = psA.tile([128, 512], F32, tag=f"pps{(2 + ci) % 4}",
                                  bufs=1, name=f"pps{(2 + ci) % 4}")
                    ci += 1
                    for c in range(KC):
                        nc.tensor.matmul(
                            ps, wcx[:, c, pj * 128:(pj + 1) * 128],
                            cxT_t[:, c, sp],
                            start=(c == 0), stop=(c == KC - 1))
                    nc.scalar.copy(out=dst[:, sp], in_=ps)

            # v_nat tiles: 4 j's per [128, 520] tile, each j = [64 vfeat h0 |
            # ones | 64 vfeat h1 | ones] so the PV stationary is contiguous.
            def v_transpose_half(src512, base):
                v_t = vnp.tile([128, 520], BF16, tag="vn", name=f"vn{base}")
                for jj in range(4):
                    vn[base + jj] = (v_t, jj)
                ps = psA.tile([128, 512], BF16, tag="tpsr", bufs=2,
                              name="tpsr")
                for jj in range(4):
                    nc.tensor.transpose(
                        ps[:, jj * 128:(jj + 1) * 128],
                        src512[:, jj * 128:(jj + 1) * 128], identb)
                nc.gpsimd.tensor_copy(
                    out=v_t.rearrange("p (a b) -> p a b", b=65)[:, :, 64:65],
                    in_=ones_col2.rearrange("p (a b) -> p a b", b=1))
                nc.vector.tensor_copy(
                    out=v_t.rearrange("p (a b) -> p a b", b=65)[:, :, 0:64],
                    in_=ps.rearrange("p (a b) -> p a b", b=64))

            v_transpose_half(vinTg[0], 8)
            v_transpose_half(vinTg[1], 12)
            v_transpose_half(vcxT[:, 0:512], 0)
            v_transpose_half(vcxT[:, 512:1024], 4)

            # dummy exp: forces the Exp act-table load off the attention
            # start (the load costs ~1.3us on ACT)
            junk = tiny.tile([128, 1], BF16, tag="junk", name="junk")
            nc.scalar.activation(out=junk, in_=eps_col, func=AF.Exp)

            if phase == 1:
                for t, src_t in enumerate((qTg[0], kinTg[0], vinTg[0],
                                           kcxT[:, 0:512], vcxT[:, 0:512],
                                           qTg[1], kinTg[1], vinTg[1])):
                    nc.sync.dma_start(
                        out=o_d[t * 128:(t + 1) * 128, 0:512].bitcast(BF16),
                        in_=src_t)
                return

        # ---- attention + final projection ----
        with tc.tile_pool(name="psSim", bufs=1, space="PSUM") as psS, \
             tc.tile_pool(name="psO", bufs=1, space="PSUM") as psO, \
             tc.tile_pool(name="psF", bufs=1, space="PSUM") as psF:
            pend_final = [None]

            def final_head(g, o_ps):
                """lrec/lbc/oT chain. MUST be fully emitted before the next
                g's first PV (o_ps ring reuse is ordered by emission)."""
                lrec = [tiny.tile([1, 512], BF16, tag=f"lr{h}", bufs=2,
                                  name=f"lr{h}") for h in (0, 1)]
                with nc.allow_low_precision(reason="1/l in bf16 is plenty"):
                    for h in (0, 1):
                        nc.vector.reciprocal(out=lrec[h],
                                             in_=o_ps[h][64:65, :])
                lbc_ps = psF.tile([128, 512], F32, tag="fin0", bufs=1,
                                  name="lbc")
                for h in (0, 1):
                    nc.tensor.matmul(
                        lbc_ps, sel2[:, 128 * h:128 * h + 128], lrec[h],
                        start=(h == 0), stop=(h == 1))
                lbc = tiny.tile([128, 512], F32, tag="lbc", bufs=2,
                                name="lbc")
                nc.vector.tensor_copy(out=lbc, in_=lbc_ps)
                oT = otp.tile([128, 512], BF16, tag="oT")
                for h in (0, 1):
                    nc.vector.tensor_tensor(
                        out=oT[64 * h:64 * h + 64, :], in0=o_ps[h][0:64, :],
                        in1=lbc[64 * h:64 * h + 64, :], op=ALU.mult)
                pend_final[0] = None
                return oT

            def fin_tile(g, oT, t, tail):
                """Out-projection + store for one 128-token tile."""
                o_r = o_d.rearrange("(t p) d -> p t d", p=128)
                ost = ostp.tile([128, 1, DIM], BF16, tag="ost")
                for half in range(2):
                    wsp = slice(half * 512, (half + 1) * 512)
                    fp = psF.tile([128, 512], F32, tag=f"fin{half}",
                                  bufs=1, name=f"fin{half}")
                    nc.tensor.matmul(
                        fp, oT[:, t * 128:(t + 1) * 128], wo[:, wsp],
                        start=True, stop=True)
                    # at the tail ACT is idle (no more exp) — alternate
                    if tail and half == 1:
                        nc.scalar.copy(out=ost[:, 0, wsp], in_=fp)
                    else:
                        nc.vector.tensor_copy(out=ost[:, 0, wsp], in_=fp)
                eng = nc.sync if (not tail or t % 2 == 0) else nc.scalar
                eng.dma_start(out=o_r[:, g * 4 + t:g * 4 + t + 1, :], in_=ost)

            # overlay work: q/kin g1 chains (PSUM: the idle fin tags) stream
            # into attention-g0's exp-wait gaps; final-g0's out-projection
            # tiles stream into attention-g1's.
            overlay = (in_chain(winq, qTg[1], 1, psF, "fin0", 1)
                       + in_chain(wink, kinTg[1], 1, psF, "fin1", 1))

            def final_half(o_ps, ca, cb, tiles, tail):
                """Normalize + out-project token cols [ca:cb] (g=1 halves)."""
                w = cb - ca
                lrec = [tiny.tile([1, 512], BF16, tag=f"lr{h}", bufs=2,
                                  name=f"lr{h}") for h in (0, 1)]
                with nc.allow_low_precision(reason="1/l in bf16 is plenty"):
                    for h in (0, 1):
                        nc.vector.reciprocal(out=lrec[h][:, 0:w],
                                             in_=o_ps[h][64:65, ca:cb])
                lbc_ps = psF.tile([128, 512], F32, tag="fin0", bufs=1,
                                  name="lbc")
                for h in (0, 1):
                    nc.tensor.matmul(
                        lbc_ps[:, ca:cb], sel2[:, 128 * h:128 * h + 128],
                        lrec[h][:, 0:w], start=(h == 0), stop=(h == 1))
                lbc = tiny.tile([128, 512], F32, tag="lbc", bufs=2,
                                name="lbc")
                nc.vector.tensor_copy(out=lbc[:, ca:cb], in_=lbc_ps[:, ca:cb])
                oT = otp.tile([128, 512], BF16, tag="oT")
                for h in (0, 1):
                    nc.vector.tensor_tensor(
                        out=oT[64 * h:64 * h + 64, ca:cb],
                        in0=o_ps[h][0:64, ca:cb],
                        in1=lbc[64 * h:64 * h + 64, ca:cb], op=ALU.mult)
                for t in tiles:
                    fin_tile(1, oT, t, tail=tail)

            prev_g = [None]
            for g in (0, 1):
                # g0: cx0..6, in0..3, cx7 (stop on the full final span).
                # g1: cx0..6, in0..5, cx7, in6, in7 — token cols [0:256] are
                # fully accumulated at cx7, so the out-projection for token
                # tiles 0,1 runs while in6/in7 still stream (smaller tail).
                j_list = [("cx", j) for j in range(7)]
                if g == 0:
                    j_list += [("in", j) for j in range(4)]
                    j_list.append(("cx", 7))
                else:
                    j_list += [("in", j) for j in range(6)]
                    j_list += [("cx", 7), ("in", 6), ("in", 7)]
                n_j = len(j_list)
                o_ps = [psO.tile([128, 512], F32, tag=f"o{h}", name=f"ops{h}")
                        for h in (0, 1)]

                def j_meta(idx, g=g, j_list=j_list):
                    src, j = j_list[idx]
                    if src == "cx":
                        return kcxT, j, j, 0, False
                    off = max(0, 128 * (j - 4 * g))
                    return None, j, 8 + j, off, j >= 4 * g

                sims = [None] * n_j

                def emit_sim(idx, j_meta=j_meta, sims=sims, g=g):
                    kT, j, jg, off, diag = j_meta(idx)
                    ps = psS.tile([128, 1024], F32, tag="sim", bufs=2,
                                  name="sim")
                    for h in (0, 1):
                        hsl = slice(64 * h, 64 * h + 64)
                        if kT is None:
                            kop = kinTg[j // 4][hsl, (j % 4) * 128:
                                                (j % 4) * 128 + 128]
                        else:
                            kop = kT[hsl, j * 128:(j + 1) * 128]
                        nc.tensor.matmul(
                            ps[:, 512 * h + off:512 * (h + 1)],
                            kop, qTg[g][hsl, off:512],
                            start=True, stop=True)
                    sims[idx] = ps

                # software pipeline: sim for j+1 is emitted before PV of j so
                # the in-order PE computes the next sim while ACT runs exp.
                emit_sim(0)
                fin_steps = []
                if pend_final[0] is not None:
                    oT_prev = final_head(prev_g[0], pend_final[0])
                    fin_steps = [(prev_g[0], oT_prev, t) for t in range(4)]
                for idx in range(n_j):
                    if idx + 1 < n_j:
                        emit_sim(idx + 1)
                    if fin_steps and idx >= 2 and idx % 2 == 0:
                        pg, oTp, t = fin_steps.pop(0)
                        fin_tile(pg, oTp, t, tail=False)
                    for _ in range(2):
                        if overlay:
                            overlay.pop(0)()
                    kT, j, jg, off, diag = j_meta(idx)
                    p_t = ppool.tile([128, 1024], BF16, tag="p", name="p")
                    ps3 = sims[idx].rearrange("p (h t) -> p h t", h=2)
                    p3 = p_t.rearrange("p (h t) -> p h t", h=2)
                    nc.scalar.activation(
                        out=p3[:, :, off:512], in_=ps3[:, :, off:512],
                        func=AF.Exp)
                    if diag:
                        for h in (0, 1):
                            nc.gpsimd.tensor_tensor(
                                out=p_t[:, 512 * h + off:512 * h + off + 128],
                                in0=p_t[:, 512 * h + off:512 * h + off + 128],
                                in1=tri, op=ALU.mult)
                    sims[idx] = None
                    v_t, jj = vn[jg]
                    if g == 0:
                        spans = [(off, 512, idx == n_j - 1)]
                    elif idx == 13:    # cx7: [0:256] complete
                        spans = [(0, 256, True), (256, 512, False)]
                    elif idx == 14:    # in6 (off 256): [256:384] complete
                        spans = [(256, 384, True), (384, 512, False)]
                    elif idx == 15:    # in7 (off 384)
                        spans = [(384, 512, True)]
                    else:
                        spans = [(off, 512, False)]
                    for lo, hi, stop in spans:
                        for h in (0, 1):
                            nc.tensor.matmul(
                                o_ps[h][0:65, lo:hi],
                                v_t[:, 130 * jj + 65 * h:
                                    130 * jj + 65 * h + 65],
                                p_t[:, 512 * h + lo:512 * h + hi],
                                start=(idx == 0), stop=stop)
                    if g == 1 and idx == 13:
                        final_half(o_ps, 0, 256, (0, 1), tail=False)
                for th in overlay:
                    th()
                overlay = []
                for pg, oTp, t in fin_steps:
                    fin_tile(pg, oTp, t, tail=False)
                pend_final[0] = o_ps
                prev_g[0] = g
            final_half(pend_final[0], 256, 512, (2, 3), tail=True)


_NC_CACHE = None


def _get_nc():
    global _NC_CACHE
    if _NC_CACHE is None:
        _NC_CACHE = build_program()
    return _NC_CACHE


def make_in_maps(x, context, gamma, beta, Wq, Wkv, Wo, bo):
    import ml_dtypes
    BF = ml_dtypes.bfloat16
    x = np.asarray(x, np.float32)
    context = np.asarray(context, np.float32)
    gamma = np.asarray(gamma, np.float32)
    beta = np.asarray(beta, np.float32)
    Wq = np.asarray(Wq, np.float32)
    Wkv = np.asarray(Wkv, np.float32)
    Wo = np.asarray(Wo, np.float32)

    s = DH ** -0.5
    in_maps = []
    for core in range(8):
        b, hg = divmod(core, 4)
        cols = slice(128 * hg, 128 * hg + 128)
        wq = Wq[:, cols] * gamma[:, None] * s
        uq = wq.sum(0)
        bq = beta @ Wq[:, cols] * s
        wk = Wkv[:, :INNER][:, cols] * gamma[:, None]
        uk = wk.sum(0)
        bk = beta @ Wkv[:, :INNER][:, cols]
        wv = Wkv[:, INNER:][:, cols] * gamma[:, None]
        uv = wv.sum(0)
        bv = beta @ Wkv[:, INNER:][:, cols]

        # per-projection 9-chunk blocks (chunk 8 = aug rows u, b)
        def blk(w, u, bvec):
            out = np.zeros((128, KC + 1, 128), np.float32)
            for c in range(KC):
                out[:, c, :] = w[128 * c:128 * c + 128]
            out[0, KC, :] = u
            out[1, KC, :] = bvec
            return out.reshape(128, 1152)

        wcx = np.zeros((128, KC, 256), np.float32)
        for c in range(KC):
            rows = slice(128 * c, 128 * c + 128)
            wcx[:, c, 0:128] = Wkv[:, :INNER][rows, cols]
            wcx[:, c, 128:256] = Wkv[:, INNER:][rows, cols]

        b1 = np.zeros((128, B1_COLS), np.float32)
        b1[:, B1_WCX:B1_WCX + 2048] = wcx.reshape(128, 2048)
        b1[:, B1_IDB:B1_IDB + 128] = np.eye(128, dtype=np.float32)
        b1[:, B1_TRI:B1_TRI + 128] = np.tril(np.ones((128, 128), np.float32)).T
        b1[2, B1_PICK:B1_PICK + 128] = 1.0
        b1[0, B1_SEL:B1_SEL + 64] = 1.0
        b1[0, B1_SEL + 192:B1_SEL + 256] = 1.0

        b2 = np.zeros((128, B2_COLS), np.float32)
        b2[:, B2_WQ:B2_WQ + 1152] = blk(wq, uq, bq)
        b2[:, B2_WK:B2_WK + 1152] = blk(wk, uk, bk)
        b2[:, B2_WV:B2_WV + 1152] = blk(wv, uv, bv)
        b2[:, B2_WO:B2_WO + 1024] = Wo[cols, :]

        in_maps.append({
            "x": np.ascontiguousarray(x[b]).astype(BF),
            "cx": np.ascontiguousarray(context[b]).astype(BF),
            "b1": b1.astype(BF),
            "b2": b2.astype(BF),
        })
    return in_maps


def assemble(results, bo):
    bo = np.asarray(bo, np.float32)
    out = np.zeros((B, N, DIM), np.float32)
    for core in range(8):
        b = core // 4
        out[b] += results[core]["o"].astype(np.float32)
    out += bo[None, None, :]
    return out


def kernel(x, context, gamma, beta, Wq, Wkv, Wo, bo):
    nc = _get_nc()
    in_maps = make_in_maps(x, context, gamma, beta, Wq, Wkv, Wo, bo)
    res = run_bass_kernel_spmd(nc, in_maps, list(range(8)))
    return assemble(res.results, bo)


# revision 45
# speedup vs baseline: 1.5226x; 1.0079x over previous
"""Trainium2 Bass kernel for nn_CausalPrefixAttention (8-core SPMD), v3.1.

Changes vs v2 (119.6us):
  - cx is never loaded in natural layout: 8 XBAR DMA-transposes load cxT
    straight from HBM into SBUF, removing 64 PE transposes and 8 big
    PSUM->SBUF copies. ALL XBAR transposes share one queue: two concurrent
    XBAR DMAs on different queues corrupt each other (measured on device;
    per-16-token stripes of garbage). Regular DMAs on other queues are ok.
  - x still loads natural (bn_stats needs tokens-on-partitions); PE
    transposes it per-tile during the otherwise DMA-bound head (first PSUM
    batch needs only x tile 0), with all 8 PSUM->SBUF copies on ACT
    (idle then) and stats on DVE.
  - weights+consts packed into blob DMAs; win's q-block is a separate DMA
    so the q projection can start before the k/v blocks land.
  - sim PSUM is one [128,1024] f32 2-bank tile per j-tile (h0|h1), so exp
    is a single strided ACT instruction per j-tile instead of two.
  - causal tri-masking on gpsimd (Pool); out-projection PSUM->SBUF copies
    on DVE, keeping ACT = pure exp during attention.
  - final: both heads' 1/l in one reciprocal + one sel-matmul.
  - emission order (x-T, stats, cx-proj, q/k/v-proj, attention) matches
    DMA arrival so the in-order PE rarely stalls: the cost model halves PE
    clock for 3us after every stall.
"""

import os
import sys

for _p in ("/opt/trn_rl_repo", "/root/.axon_site/_ro/trn_rl_repo"):
    if os.path.isdir(_p) and _p not in sys.path:
        sys.path.append(_p)

import numpy as np

import concourse.mybir as mybir
import concourse.tile as tile
from concourse import bacc
from concourse.bass_utils import run_bass_kernel_spmd

F32 = mybir.dt.float32
BF16 = mybir.dt.bfloat16
AF = mybir.ActivationFunctionType
ALU = mybir.AluOpType

B, N, M, DIM, INNER, HEADS, DH = 2, 1024, 1024, 1024, 512, 8, 64
EPS = 1e-5
NT = N // 128      # token tiles per batch (8)
KC = DIM // 128    # contraction chunks (8)

# blob1 column offsets (bf16): wcx | idb | tri | pick | sel (row 0, 2x128)
B1_WCX, B1_IDB, B1_TRI, B1_PICK, B1_SEL = 0, 2048, 2176, 2304, 2432
B1_COLS = 2688
# blob2: win q-block | k-block | v-block | wo (split DMA: q early, rest later)
B2_WQ, B2_WK, B2_WV, B2_WO = 0, 1152, 2304, 3456
B2_COLS = 4480


def build_program(unroll=1, phase=2):
    nc = bacc.Bacc("TRN2", target_bir_lowering=False, debug=False)

    x_d = nc.dram_tensor("x", [N, DIM], BF16, kind="ExternalInput")
    cx_d = nc.dram_tensor("cx", [M, DIM], BF16, kind="ExternalInput")
    b1_d = nc.dram_tensor("b1", [128, B1_COLS], BF16, kind="ExternalInput")
    b2_d = nc.dram_tensor("b2", [128, B2_COLS], BF16, kind="ExternalInput")
    o_d = nc.dram_tensor("o", [N, DIM], BF16, kind="ExternalOutput")

    with tile.TileContext(nc) as tc:
        for _ in range(unroll):
            _emit(nc, tc, x_d, cx_d, b1_d, b2_d, o_d, phase)
    nc.compile()
    return nc


def _emit(nc, tc, x_d, cx_d, b1_d, b2_d, o_d, phase=2):
    from contextlib import ExitStack

    ctx = ExitStack()
    with ctx:
        wpool = ctx.enter_context(tc.tile_pool(name="wpool", bufs=1))
        projp = ctx.enter_context(tc.tile_pool(name="projp", bufs=8))
        vnp = ctx.enter_context(tc.tile_pool(name="vnp", bufs=4))
        ppool = ctx.enter_context(tc.tile_pool(name="ppool", bufs=5))
        otp = ctx.enter_context(tc.tile_pool(name="otp", bufs=2))
        ostp = ctx.enter_context(tc.tile_pool(name="ostp", bufs=4))
        tiny = ctx.enter_context(tc.tile_pool(name="tiny", bufs=8))
        consts = ctx.enter_context(tc.tile_pool(name="consts", bufs=1))

        eps_col = consts.tile([128, 1], F32)
        nc.vector.memset(eps_col, EPS)
        ones_col2 = consts.tile([128, 8], BF16)
        nc.vector.memset(ones_col2, 1.0)

        # ---- input DMA stream. DMA issue costs ~1.2us each on the HWDGE
        # queues (SEQ+HWDGE) and ACT-queue issues block ACT engine work, so:
        # sync queue = x pair-loads + the 8 XBAR transposes (consumption
        # order); Pool/SWDGE queue = all weight/const blobs (desc-gen runs
        # on the idle Pool engine, 25ns SEQ). Scalar issues nothing early.
        b1 = wpool.tile([128, B1_COLS], BF16, tag="b1")
        b2 = wpool.tile([128, B2_COLS], BF16, tag="b2")
        natx = ctx.enter_context(tc.tile_pool(name="natx", bufs=1))
        xnat_t = natx.tile([128, NT, DIM], BF16, tag="nat", name="xnat")
        x_r = x_d.rearrange("(t p) d -> p t d", p=128)
        for hp in range(NT // 2):
            nc.sync.dma_start(out=xnat_t[:, 2 * hp:2 * hp + 2, :],
                              in_=x_r[:, 2 * hp:2 * hp + 2, :])
        x_nat = [xnat_t[:, t, :] for t in range(NT)]

        # cxT via XBAR DMA transpose, chunk-major (single queue — see above)
        cxT_t = wpool.tile([128, KC, M], BF16, tag="cxT")
        for c in range(KC):
            nc.sync.dma_start(out=cxT_t[:, c, :],
                              in_=cx_d[:, c * 128:(c + 1) * 128],
                              transpose=True)

        nc.gpsimd.dma_start(out=b1[:, B1_IDB:], in_=b1_d[:, B1_IDB:])
        nc.gpsimd.dma_start(out=b2[:, 0:B2_WK], in_=b2_d[:, 0:B2_WK])
        nc.gpsimd.dma_start(out=b2[:, B2_WK:B2_WV], in_=b2_d[:, B2_WK:B2_WV])
        nc.gpsimd.dma_start(out=b1[:, 0:B1_IDB], in_=b1_d[:, 0:B1_IDB])
        nc.gpsimd.dma_start(out=b2[:, B2_WV:], in_=b2_d[:, B2_WV:])
        wcx = b1[:, B1_WCX:B1_WCX + 2048].rearrange("p (c k) -> p c k", k=256)
        identb = b1[:, B1_IDB:B1_IDB + 128]
        tri = b1[:, B1_TRI:B1_TRI + 128]
        pick3 = b1[0:3, B1_PICK:B1_PICK + 128]
        sel2 = b1[0:1, B1_SEL:B1_SEL + 256]
        winq = b2[:, B2_WQ:B2_WQ + 1152].rearrange("p (c k) -> p c k", k=128)
        wink = b2[:, B2_WK:B2_WK + 1152].rearrange("p (c k) -> p c k", k=128)
        winv = b2[:, B2_WV:B2_WV + 1152].rearrange("p (c k) -> p c k", k=128)
        wo = b2[:, B2_WO:B2_WO + 1024]
        # stat rows: row0 = -mu, row1 = std (aug contraction), row2 = rs
        srow = consts.tile([3, N], BF16)

        # per-token-half projection tiles: no false write-after-read deps
        # when the g1-half chains stream into attention-g0
        kcxT = projp.tile([128, M], BF16, tag="proj", name="kcxT")
        vcxT = projp.tile([128, M], BF16, tag="proj", name="vcxT")
        qTg = [projp.tile([128, 512], BF16, tag="proj", name=f"qT{g}")
               for g in range(2)]
        kinTg = [projp.tile([128, 512], BF16, tag="proj", name=f"kinT{g}")
                 for g in range(2)]
        vinTg = [projp.tile([128, 512], BF16, tag="proj", name=f"vinT{g}")
                 for g in range(2)]
        rsb = ctx.enter_context(tc.tile_pool(name="rsb", bufs=2))
        rs_bc = [rsb.tile([128, 512], F32, tag="rsbc", name=f"rsbc{g}")
                 for g in range(2)]
        vn = [None] * 16

        phase_a = ExitStack()
        with phase_a:
            tposed = phase_a.enter_context(tc.tile_pool(name="tposed", bufs=1))
            psA = phase_a.enter_context(
                tc.tile_pool(name="psA", bufs=1, space="PSUM"))

            # ---- x transposes on PE, one x-tile per PSUM batch so the
            # first batch only needs x tile 0; copies on ACT; bn_stats on
            # DVE per tile, post-processing batched at the end ----
            xT = tposed.tile([128, 2, KC, 512], BF16, tag="tp", name="xT")
            s4a = tiny.tile([128, NT, 4], F32, tag="s4a", name="s4a")
            for t in range(NT):
                ps = psA.tile([128, 1024], BF16, tag="tps", bufs=2,
                              name="tps")
                for c in range(KC):
                    nc.tensor.transpose(
                        ps[:, c * 128:(c + 1) * 128],
                        x_nat[t][:, c * 128:(c + 1) * 128], identb)
                co = (t % 4) * 128
                nc.scalar.copy(
                    out=xT[:, t // 4, :, co:co + 128],
                    in_=ps.rearrange("p (c k) -> p c k", k=128))
                bst = tiny.tile([128, 2, 6], F32, tag="bst", name="bst")
                for half in range(2):
                    nc.vector.bn_stats(
                        out=bst[:, half, :],
                        in_=x_nat[t][:, half * 512:(half + 1) * 512])
                nc.vector.bn_aggr(out=s4a[:, t, 0:2], in_=bst)

            # batched stats post-processing: cols 0:2 = (mu, var) per tile;
            # -> col 0 = -mu, col 1 = std, col 2 = rs
            nc.scalar.activation(
                out=s4a.rearrange("p t k -> p (t k)")[:, 1::4],
                in_=s4a.rearrange("p t k -> p (t k)")[:, 1::4],
                func=AF.Sqrt, bias=eps_col)
            nc.vector.reciprocal(
                out=s4a.rearrange("p t k -> p (t k)")[:, 2::4],
                in_=s4a.rearrange("p t k -> p (t k)")[:, 1::4])
            nc.vector.tensor_scalar(
                out=s4a.rearrange("p t k -> p (t k)")[:, 0::4],
                in0=s4a.rearrange("p t k -> p (t k)")[:, 0::4],
                scalar1=-1.0, scalar2=None, op0=ALU.mult)
            s4b = tiny.tile([128, NT, 3], BF16, tag="s4b", name="s4b")
            nc.vector.tensor_copy(out=s4b, in_=s4a[:, :, 0:3])

            def in_chain(w9, dst, gg, pool, tag, bufs):
                """One input-projection half: 8 chunks + aug, rs on copy-out.
                Returns the matmul/copy thunks for interleaved emission."""
                st = {}
                sp = slice(gg * 512, (gg + 1) * 512)

                def step(c):
                    if c == 0:
                        st["ps"] = pool.tile([128, 512], F32, tag=tag,
                                             bufs=bufs, name=tag)
                    nc.tensor.matmul(
                        st["ps"], w9[:, c, :], xT[:, gg, c, :],
                        start=(c == 0), stop=False)

                def aug():
                    nc.tensor.matmul(
                        st["ps"], w9[0:2, KC, :], srow[0:2, sp],
                        start=False, stop=True)
                    nc.vector.tensor_tensor(
                        out=dst, in0=st["ps"], in1=rs_bc[gg], op=ALU.mult)

                return [lambda c=c: step(c) for c in range(KC)] + [aug]

            # projections run in pairs around the stats-row build so the
            # 4-tag PSUM ring always has 2 free banks for rs_bc/next pair
            pair_a = [in_chain(winq, qTg[0], 0, psA, "pps0", 1),
                      in_chain(wink, kinTg[0], 0, psA, "pps1", 1)]
            for ch in pair_a:
                for th in ch[:-1]:
                    th()

            # ---- stats rows (PE transposes are tiny; stats long done) ----
            for t in range(NT):
                ps2 = psA.tile([128, 512], BF16, tag="tpsr", bufs=2,
                               name="tpsr")
                nc.tensor.transpose(ps2[0:3, 0:128], s4b[:, t, :], identb)
                nc.vector.tensor_copy(
                    out=srow[:, t * 128:(t + 1) * 128], in_=ps2[0:3, 0:128])
            # rs broadcast tiles: pick3^T selects srow row 2 into every part
            for g in range(2):
                ps = psA.tile([128, 512], F32, tag=f"pps{2 + g}", bufs=1,
                              name=f"pps{2 + g}")
                nc.tensor.matmul(
                    ps, pick3, srow[:, g * 512:(g + 1) * 512],
                    start=True, stop=True)
                nc.scalar.copy(out=rs_bc[g], in_=ps)
            for ch in pair_a:
                ch[-1]()

            pair_b = [in_chain(winv, vinTg[0], 0, psA, "pps0", 1),
                      in_chain(winv, vinTg[1], 1, psA, "pps1", 1)]
            for ch in pair_b:
                for th in ch:
                    th()

            # ---- context projections (cxT streamed by the DMA queue);
            # copies on ACT ----
            ci = 0
            for pj, dst in ((0, kcxT), (1, vcxT)):
                for gg in (0, 1):
                    sp = slice(gg * 512, (gg + 1) * 512)
                    ps = psA.tile([128, 512], F32, tag=f"pps{(2 + ci) % 4}",
                                  bufs=1, name=f"pps{(2 + ci) % 4}")
                    ci += 1
                    for c in range(KC):
                        nc.tensor.matmul(
                            ps, wcx[:, c, pj * 128:(pj + 1) * 128],
                            cxT_t[:, c, sp],
                            start=(c == 0), stop=(c == KC - 1))
                    nc.scalar.copy(out=dst[:, sp], in_=ps)

            # v_nat tiles: 4 j's per [128, 520] tile, each j = [64 vfeat h0 |
            # ones | 64 vfeat h1 | ones] so the PV stationary is contiguous.
            def v_transpose_half(src512, base):
                v_t = vnp.tile([128, 520], BF16, tag="vn", name=f"vn{base}")
                for jj in range(4):
                    vn[base + jj] = (v_t, jj)
                ps = psA.tile([128, 512], BF16, tag="tpsr", bufs=2,
                              name="tpsr")
                for jj in range(4):
                    nc.tensor.transpose(
                        ps[:, jj * 128:(jj + 1) * 128],
                        src512[:, jj * 128:(jj + 1) * 128], identb)
                nc.gpsimd.tensor_copy(
                    out=v_t.rearrange("p (a b) -> p a b", b=65)[:, :, 64:65],
                    in_=ones_col2.rearrange("p (a b) -> p a b", b=1))
                nc.vector.tensor_copy(
                    out=v_t.rearrange("p (a b) -> p a b", b=65)[:, :, 0:64],
                    in_=ps.rearrange("p (a b) -> p a b", b=64))

            v_transpose_half(vinTg[0], 8)
            v_transpose_half(vinTg[1], 12)
            v_transpose_half(vcxT[:, 0:512], 0)
            v_transpose_half(vcxT[:, 512:1024], 4)

            # dummy exp: forces the Exp act-table load off the attention
            # start (the load costs ~1.3us on ACT)
            junk = tiny.tile([128, 1], BF16, tag="junk", name="junk")
            nc.scalar.activation(out=junk, in_=eps_col, func=AF.Exp)

            if phase == 1:
                for t, src_t in enumerate((qTg[0], kinTg[0], vinTg[0],
                                           kcxT[:, 0:512], vcxT[:, 0:512],
                                           qTg[1], kinTg[1], vinTg[1])):
                    nc.sync.dma_start(
                        out=o_d[t * 128:(t + 1) * 128, 0:512].bitcast(BF16),
                        in_=src_t)
                return

        # ---- attention + final projection ----
        with tc.tile_pool(name="psSim", bufs=1, space="PSUM") as psS, \
             tc.tile_pool(name="psO", bufs=1, space="PSUM") as psO, \
             tc.tile_pool(name="psF", bufs=1, space="PSUM") as psF:
            pend_final = [None]

            def final_head(g, o_ps):
                """lrec/lbc/oT chain. MUST be fully emitted before the next
                g's first PV (o_ps ring reuse is ordered by emission)."""
                lrec = [tiny.tile([1, 512], BF16, tag=f"lr{h}", bufs=2,
                                  name=f"lr{h}") for h in (0, 1)]
                with nc.allow_low_precision(reason="1/l in bf16 is plenty"):
                    for h in (0, 1):
                        nc.vector.reciprocal(out=lrec[h],
                                             in_=o_ps[h][64:65, :])
                lbc_ps = psF.tile([128, 512], F32, tag="fin0", bufs=1,
                                  name="lbc")
                for h in (0, 1):
                    nc.tensor.matmul(
                        lbc_ps, sel2[:, 128 * h:128 * h + 128], lrec[h],
                        start=(h == 0), stop=(h == 1))
                lbc = tiny.tile([128, 512], F32, tag="lbc", bufs=2,
                                name="lbc")
                nc.vector.tensor_copy(out=lbc, in_=lbc_ps)
                oT = otp.tile([128, 512], BF16, tag="oT")
                for h in (0, 1):
                    nc.vector.tensor_tensor(
                        out=oT[64 * h:64 * h + 64, :], in0=o_ps[h][0:64, :],
                        in1=lbc[64 * h:64 * h + 64, :], op=ALU.mult)
                pend_final[0] = None
                return oT

            def fin_tile(g, oT, t, tail):
                """Out-projection + store for one 128-token tile."""
                o_r = o_d.rearrange("(t p) d -> p t d", p=128)
                ost = ostp.tile([128, 1, DIM], BF16, tag="ost")
                for half in range(2):
                    wsp = slice(half * 512, (half + 1) * 512)
                    fp = psF.tile([128, 512], F32, tag=f"fin{half}",
                                  bufs=1, name=f"fin{half}")
                    nc.tensor.matmul(
                        fp, oT[:, t * 128:(t + 1) * 128], wo[:, wsp],
                        start=True, stop=True)
                    # at the tail ACT is idle (no more exp) — alternate
                    if tail and half == 1:
                        nc.scalar.copy(out=ost[:, 0, wsp], in_=fp)
                    else:
                        nc.vector.tensor_copy(out=ost[:, 0, wsp], in_=fp)
                eng = nc.sync if (not tail or t % 2 == 0) else nc.scalar
                eng.dma_start(out=o_r[:, g * 4 + t:g * 4 + t + 1, :], in_=ost)

            # overlay work: q/kin g1 chains (PSUM: the idle fin tags) stream
            # into attention-g0's exp-wait gaps; final-g0's out-projection
            # tiles stream into attention-g1's.
            overlay = (in_chain(winq, qTg[1], 1, psF, "fin0", 1)
                       + in_chain(wink, kinTg[1], 1, psF, "fin1", 1))

            def final_half(o_ps, ca, cb, tiles, tail):
                """Normalize + out-project token cols [ca:cb] (g=1 halves)."""
                w = cb - ca
                lrec = [tiny.tile([1, 512], BF16, tag=f"lr{h}", bufs=2,
                                  name=f"lr{h}") for h in (0, 1)]
                with nc.allow_low_precision(reason="1/l in bf16 is plenty"):
                    for h in (0, 1):
                        nc.vector.reciprocal(out=lrec[h][:, 0:w],
                                             in_=o_ps[h][64:65, ca:cb])
                lbc_ps = psF.tile([128, 512], F32, tag="fin0", bufs=1,
                                  name="lbc")
                for h in (0, 1):
                    nc.tensor.matmul(
                        lbc_ps[:, ca:cb], sel2[:, 128 * h:128 * h + 128],
                        lrec[h][:, 0:w], start=(h == 0), stop=(h == 1))
                lbc = tiny.tile([128, 512], F32, tag="lbc", bufs=2,
                                name="lbc")
                nc.vector.tensor_copy(out=lbc[:, ca:cb], in_=lbc_ps[:, ca:cb])
                oT = otp.tile([128, 512], BF16, tag="oT")
                for h in (0, 1):
                    nc.vector.tensor_tensor(
                        out=oT[64 * h:64 * h + 64, ca:cb],
                        in0=o_ps[h][0:64, ca:cb],
                        in1=lbc[64 * h:64 * h + 64, ca:cb], op=ALU.mult)
                for t in tiles:
                    fin_tile(1, oT, t, tail=tail)

            prev_g = [None]
            for g in (0, 1):
                # g0: cx0..6, in0..3, cx7 (stop on the full final span).
                # g1: cx0..6, in0..5, cx7, in6, in7 — token cols [0:256] are
                # fully accumulated at cx7, so the out-projection for token
                # tiles 0,1 runs while in6/in7 still stream (smaller tail).
                j_list = [("cx", j) for j in range(7)]
                if g == 0:
                    j_list += [("in", j) for j in range(4)]
                    j_list.append(("cx", 7))
                else:
                    j_list += [("in", j) for j in range(6)]
                    j_list += [("cx", 7), ("in", 6), ("in", 7)]
                n_j = len(j_list)
                o_ps = [psO.tile([128, 512], F32, tag=f"o{h}", name=f"ops{h}")
                        for h in (0, 1)]

                def j_meta(idx, g=g, j_list=j_list):
                    src, j = j_list[idx]
                    if src == "cx":
                        return kcxT, j, j, 0, False
                    off = max(0, 128 * (j - 4 * g))
                    return None, j, 8 + j, off, j >= 4 * g

                sims = [None] * n_j

                def emit_sim(idx, j_meta=j_meta, sims=sims, g=g):
                    kT, j, jg, off, diag = j_meta(idx)
                    ps = psS.tile([128, 1024], F32, tag="sim", bufs=2,
                                  name="sim")
                    for h in (0, 1):
                        hsl = slice(64 * h, 64 * h + 64)
                        if kT is None:
                            kop = kinTg[j // 4][hsl, (j % 4) * 128:
                                                (j % 4) * 128 + 128]
                        else:
                            kop = kT[hsl, j * 128:(j + 1) * 128]
                        nc.tensor.matmul(
                            ps[:, 512 * h + off:512 * (h + 1)],
                            kop, qTg[g][hsl, off:512],
                            start=True, stop=True)
                    sims[idx] = ps

                # software pipeline: sim for j+1 is emitted before PV of j so
                # the in-order PE computes the next sim while ACT runs exp.
                emit_sim(0)
                fin_steps = []
                if pend_final[0] is not None:
                    oT_prev = final_head(prev_g[0], pend_final[0])
                    fin_steps = [(prev_g[0], oT_prev, t) for t in range(4)]
                for idx in range(n_j):
                    if idx + 1 < n_j:
                        emit_sim(idx + 1)
                    if fin_steps and idx >= 2 and idx % 2 == 0:
                        pg, oTp, t = fin_steps.pop(0)
                        fin_tile(pg, oTp, t, tail=False)
                    for _ in range(2):
                        if overlay:
                            overlay.pop(0)()
                    kT, j, jg, off, diag = j_meta(idx)
                    p_t = ppool.tile([128, 1024], BF16, tag="p", name="p")
                    ps3 = sims[idx].rearrange("p (h t) -> p h t", h=2)
                    p3 = p_t.rearrange("p (h t) -> p h t", h=2)
                    nc.scalar.activation(
                        out=p3[:, :, off:512], in_=ps3[:, :, off:512],
                        func=AF.Exp)
                    if diag:
                        for h in (0, 1):
                            nc.gpsimd.tensor_tensor(
                                out=p_t[:, 512 * h + off:512 * h + off + 128],
                                in0=p_t[:, 512 * h + off:512 * h + off + 128],
                                in1=tri, op=ALU.mult)
                    sims[idx] = None
                    v_t, jj = vn[jg]
                    if g == 0:
                        spans = [(off, 512, idx == n_j - 1)]
                    elif idx == 13:    # cx7: [0:256] complete
                        spans = [(0, 256, True), (256, 512, False)]
                    elif idx == 14:    # in6 (off 256): [256:384] complete
                        spans = [(256, 384, True), (384, 512, False)]
                    elif idx == 15:    # in7 (off 384)
                        spans = [(384, 512, True)]
                    else:
                        spans = [(off, 512, False)]
                    for lo, hi, stop in spans:
                        for h in (0, 1):
                            nc.tensor.matmul(
                                o_ps[h][0:65, lo:hi],
                                v_t[:, 130 * jj + 65 * h:
                                    130 * jj + 65 * h + 65],
                                p_t[:, 512 * h + lo:512 * h + hi],
                                start=(idx == 0), stop=stop)
                    if g == 1 and idx == 13:
                        final_half(o_ps, 0, 256, (0, 1), tail=False)
                for th in overlay:
                    th()
                overlay = []
                for pg, oTp, t in fin_steps:
                    fin_tile(pg, oTp, t, tail=False)
                pend_final[0] = o_ps
                prev_g[0] = g
            final_half(pend_final[0], 256, 512, (2, 3), tail=True)


_NC_CACHE = None


def _get_nc():
    global _NC_CACHE
    if _NC_CACHE is None:
        _NC_CACHE = build_program()
    return _NC_CACHE


def make_in_maps(x, context, gamma, beta, Wq, Wkv, Wo, bo):
    import ml_dtypes
    BF = ml_dtypes.bfloat16
    x = np.asarray(x, np.float32)
    context = np.asarray(context, np.float32)
    gamma = np.asarray(gamma, np.float32)
    beta = np.asarray(beta, np.float32)
    Wq = np.asarray(Wq, np.float32)
    Wkv = np.asarray(Wkv, np.float32)
    Wo = np.asarray(Wo, np.float32)

    s = DH ** -0.5
    in_maps = []
    for core in range(8):
        b, hg = divmod(core, 4)
        cols = slice(128 * hg, 128 * hg + 128)
        wq = Wq[:, cols] * gamma[:, None] * s
        uq = wq.sum(0)
        bq = beta @ Wq[:, cols] * s
        wk = Wkv[:, :INNER][:, cols] * gamma[:, None]
        uk = wk.sum(0)
        bk = beta @ Wkv[:, :INNER][:, cols]
        wv = Wkv[:, INNER:][:, cols] * gamma[:, None]
        uv = wv.sum(0)
        bv = beta @ Wkv[:, INNER:][:, cols]

        # per-projection 9-chunk blocks (chunk 8 = aug rows u, b)
        def blk(w, u, bvec):
            out = np.zeros((128, KC + 1, 128), np.float32)
            for c in range(KC):
                out[:, c, :] = w[128 * c:128 * c + 128]
            out[0, KC, :] = u
            out[1, KC, :] = bvec
            return out.reshape(128, 1152)

        wcx = np.zeros((128, KC, 256), np.float32)
        for c in range(KC):
            rows = slice(128 * c, 128 * c + 128)
            wcx[:, c, 0:128] = Wkv[:, :INNER][rows, cols]
            wcx[:, c, 128:256] = Wkv[:, INNER:][rows, cols]

        b1 = np.zeros((128, B1_COLS), np.float32)
        b1[:, B1_WCX:B1_WCX + 2048] = wcx.reshape(128, 2048)
        b1[:, B1_IDB:B1_IDB + 128] = np.eye(128, dtype=np.float32)
        b1[:, B1_TRI:B1_TRI + 128] = np.tril(np.ones((128, 128), np.float32)).T
        b1[2, B1_PICK:B1_PICK + 128] = 1.0
        b1[0, B1_SEL:B1_SEL + 64] = 1.0
        b1[0, B1_SEL + 192:B1_SEL + 256] = 1.0

        b2 = np.zeros((128, B2_COLS), np.float32)
        b2[:, B2_WQ:B2_WQ + 1152] = blk(wq, uq, bq)
        b2[:, B2_WK:B2_WK + 1152] = blk(wk, uk, bk)
        b2[:, B2_WV:B2_WV + 1152] = blk(wv, uv, bv)
        b2[:, B2_WO:B2_WO + 1024] = Wo[cols, :]

        in_maps.append({
            "x": np.ascontiguousarray(x[b]).astype(BF),
            "cx": np.ascontiguousarray(context[b]).astype(BF),
            "b1": b1.astype(BF),
            "b2": b2.astype(BF),
        })
    return in_maps


def assemble(results, bo):
    bo = np.asarray(bo, np.float32)
    out = np.zeros((B, N, DIM), np.float32)
    for core in range(8):
        b = core // 4
        out[b] += results[core]["o"].astype(np.float32)
    out += bo[None, None, :]
    return out


def kernel(x, context, gamma, beta, Wq, Wkv, Wo, bo):
    nc = _get_nc()
    in_maps = make_in_maps(x, context, gamma, beta, Wq, Wkv, Wo, bo)
    res = run_bass_kernel_spmd(nc, in_maps, list(range(8)))
    return assemble(res.results, bo)
